# revision 15
# baseline (speedup 1.0000x reference)
"""Trainium2 Bass kernel for nn_AttenuationToRainRate (dense_mlp).

Architecture notes
------------------
Reference computation per (sample b, position t):
  style MLP: metadata (16) -> 64 -> 128 -> 64, split into 4 x (scale, bias)[8]
  main chain: x -> [w1] -> adain/lrelu -> [w2] -> adain/lrelu -> [w3] ->
              adain/lrelu -> [w4] -> adain/lrelu -> [w5] -> lrelu
  adain(h) = scale * (h - mean_c h) / (std_ddof1 + 1e-6) + bias

Key algebraic transform (deferred normalization): lrelu is positively
homogeneous, and the channel-normalization d/sigma is invariant to any
positive per-position scaling of d.  We therefore never divide by sigma
inside the chain; instead we track activations scaled by lambda = sigma~
(the unnormalized std) and fold the division into:
  - per-layer bias terms, realized as PE rank-1 accumulates (b' (x) lambda)
  - the adain affine, realized as Z' = d~ + (bias/scale) (x) sigma~ followed
    by one ACT pass: a~ = Lrelu(scale * Z') (= sigma~ * lrelu(z), exact)
  - a single reciprocal at the very end: out = lrelu(h5~) * (1/sigma4~)
Mean-removal is folded into the weights host-side: W' = W (I - J/8).

Data layout per core (32 samples, data-parallel over 8 cores):
  tile [128, 512]: partition p = 8*s' + c (16 samples x 8 channels),
  free = 512 consecutive positions.  Per supergroup sg (16 samples) and
  cohort k, four tiles tau cover 2048 positions.  sigma~^2 for the 4 tiles
  of a cohort is gathered (via banded PE stationaries) into one packed
  [64, 512] psum tile (partition q = 16*tau + s'), so the ln/exp sigma chain
  runs once per cohort instead of once per tile.
"""

import numpy as np

B_FULL, T = 256, 8192
NCORES = 8
BS = B_FULL // NCORES  # 32 samples per core
F = 16

# config switches (test.py may flip these and call _reset())
CFG = {
    "dsq_dve": True,   # d~^2 on DVE via tensor_tensor(d,d) from PSUM; else ACT Square
    "lrelu": True,     # use ACT Lrelu(scale*Z') ; else abs-form (u*Z' + w*|Z'|)
}

_CACHE = {}


def _reset():
    _CACHE.clear()


# ----------------------------------------------------------------- host side

def _host_weights(inp):
    """Build all weight-derived constant tensors (f32 numpy, device layouts)."""
    f64 = np.float64
    I8 = np.eye(8, dtype=f64)
    C = I8 - np.full((8, 8), 1.0 / 8.0, dtype=f64)  # output-centering

    w = {}
    w1 = np.asarray(inp["w1"], dtype=f64)           # (1, 8)
    b1 = np.asarray(inp["b1"], dtype=f64)           # (8,)
    w1p = (w1 @ C)[0]
    b1p = b1 - b1.mean()

    w1b = np.zeros((16, 128), dtype=f64)
    b1row = np.zeros((1, 128), dtype=f64)
    for s in range(16):
        w1b[s, 8 * s:8 * s + 8] = w1p
        b1row[0, 8 * s:8 * s + 8] = b1p
    w["w1b"] = w1b
    w["b1row"] = b1row

    blamb = np.zeros((64, 12 * 128), dtype=f64)
    for l in (2, 3, 4):
        W = np.asarray(inp[f"w{l}"], dtype=f64) @ C   # (8, 8) in->out
        bp = np.asarray(inp[f"b{l}"], dtype=f64)
        bp = bp - bp.mean()
        wb = np.zeros((128, 128), dtype=f64)
        for s in range(16):
            wb[8 * s:8 * s + 8, 8 * s:8 * s + 8] = W
        w[f"wb{l}"] = wb
        for tau in range(4):
            v = (l - 2) * 4 + tau
            blk = blamb[:, 128 * v:128 * (v + 1)]
            for s in range(16):
                blk[16 * tau + s, 8 * s:8 * s + 8] = bp
    w["blamb"] = blamb

    gath = np.zeros((128, 4 * 64), dtype=f64)
    w5b = np.zeros((128, 4 * 64), dtype=f64)
    w5 = np.asarray(inp["w5"], dtype=f64)[:, 0]      # (8,)
    for tau in range(4):
        for s in range(16):
            for c in range(8):
                gath[8 * s + c, 64 * tau + 16 * tau + s] = 1.0 / 7.0
                w5b[8 * s + c, 64 * tau + 16 * tau + s] = w5[c]
    w["gath"] = gath
    w["w5b"] = w5b
    w["b5i"] = np.eye(64, dtype=f64) * float(np.asarray(inp["b5"], dtype=f64)[0])

    w["mw1"] = np.asarray(inp["mw1"], dtype=f64)
    w["mw2"] = np.asarray(inp["mw2"], dtype=f64)
    w["mw3"] = np.asarray(inp["mw3"], dtype=f64)
    w["mb1c"] = np.asarray(inp["mb1"], dtype=f64).reshape(64, 1)
    w["mb2c"] = np.asarray(inp["mb2"], dtype=f64).reshape(128, 1)
    w["mb3c"] = np.asarray(inp["mb3"], dtype=f64).reshape(64, 1)

    return {k: np.ascontiguousarray(v, dtype=np.float32) for k, v in w.items()}


_WSHAPES = {
    "w1b": [16, 128], "b1row": [1, 128],
    "wb2": [128, 128], "wb3": [128, 128], "wb4": [128, 128],
    "blamb": [64, 1536], "gath": [128, 256], "w5b": [128, 256], "b5i": [64, 64],
    "mw1": [16, 64], "mw2": [64, 128], "mw3": [128, 64],
    "mb1c": [64, 1], "mb2c": [128, 1], "mb3c": [64, 1],
}


# --------------------------------------------------------------- device side

def build_program(cfg=None):
    import concourse.bacc as bacc
    import concourse.mybir as mybir
    from concourse.ap import AP
    from concourse.tile import TileContext

    cfg = dict(CFG if cfg is None else cfg)
    f32 = mybir.dt.float32
    AF = mybir.ActivationFunctionType
    OP = mybir.AluOpType

    nc = bacc.Bacc("TRN2", target_bir_lowering=False)
    x_d = nc.dram_tensor("x", [BS, T], f32, kind="ExternalInput")
    md_d = nc.dram_tensor("metadata", [BS, F], f32, kind="ExternalInput")
    y_d = nc.dram_tensor("y", [BS, T], f32, kind="ExternalOutput")
    wd = {name: nc.dram_tensor(name, shp, f32, kind="ExternalInput")
          for name, shp in _WSHAPES.items()}

    with TileContext(nc) as tc:
        with tc.tile_pool(name="const", bufs=1) as cp, \
             tc.tile_pool(name="scr", bufs=1, space="DRAM") as dp:

            # ---- constants to SBUF
            cw = {}
            for name, shp in _WSHAPES.items():
                t = cp.tile(shp, f32, name=f"c_{name}")
                nc.sync.dma_start(out=t[:], in_=wd[name][:])
                cw[name] = t
            ones_s = cp.tile([1, 512], f32, name="ones_s")
            nc.vector.memset(ones_s[:], 1.0)
            eps_s = cp.tile([64, 1], f32, name="eps_s")
            nc.vector.memset(eps_s[:], 1e-12)

            # ---- style MLP (per-core 32 samples)
            with tc.tile_pool(name="stp", bufs=1, space="PSUM") as sp:
                mdT = cp.tile([F, BS], f32, name="mdT")
                nc.sync.dma_start(out=mdT[:], in_=md_d.rearrange("s f -> f s"))
                ps1 = sp.tile([64, BS], f32, name="ps1")
                nc.tensor.matmul(ps1[:], cw["mw1"][:], mdT[:], start=True, stop=True)
                s1 = cp.tile([64, BS], f32, name="s1")
                nc.scalar.activation(s1[:], ps1[:], AF.Relu, bias=cw["mb1c"][:])
                ps2 = sp.tile([128, BS], f32, name="ps2")
                nc.tensor.matmul(ps2[:], cw["mw2"][:], s1[:], start=True, stop=True)
                s2 = cp.tile([128, BS], f32, name="s2")
                nc.scalar.activation(s2[:], ps2[:], AF.Relu, bias=cw["mb2c"][:])
                ps3 = sp.tile([64, BS], f32, name="ps3")
                nc.tensor.matmul(ps3[:], cw["mw3"][:], s2[:], start=True, stop=True)
                sT = cp.tile([64, BS], f32, name="sT")
                nc.scalar.activation(sT[:], ps3[:], AF.Identity, bias=cw["mb3c"][:])

            # ---- style-derived constants via DRAM round-trips
            # sT[16(l-1)+2c + (0:scale,1:bias), 16 sg + s']
            sT_d = dp.tile([64, BS], f32, name="sT_d")
            nc.gpsimd.dma_start(out=sT_d[:], in_=sT[:])

            # scale vectors [128, 8]: col j = (l-1)*2+sg ; partition 8 s' + c
            scv = cp.tile([128, 8], f32, name="scv")
            for l in range(1, 5):
                for g in range(2):
                    j = (l - 1) * 2 + g
                    src = AP(tensor=sT_d[:].tensor,
                             offset=512 * (l - 1) + 16 * g,
                             ap=((1, 16), (64, 8)))
                    nc.gpsimd.dma_start(out=scv[:, j:j + 1], in_=src)

            # scale/bias as [32, 32] (row q = 8(l-1)+c)
            sc_T = cp.tile([32, BS], f32, name="sc_T")
            nc.gpsimd.dma_start(
                out=sc_T[:],
                in_=sT_d[:].rearrange("(m two) s -> two m s", two=2)[0])
            b_T = cp.tile([32, BS], f32, name="b_T")
            nc.gpsimd.dma_start(
                out=b_T[:],
                in_=sT_d[:].rearrange("(m two) s -> two m s", two=2)[1])

            # ratio = bias/scale with sign-preserving clamp of |scale|>=1e-20
            sgn = cp.tile([32, BS], f32, name="sgn")
            nc.scalar.sign(sgn[:], sc_T[:])
            absS = cp.tile([32, BS], f32, name="absS")
            nc.scalar.activation(absS[:], sc_T[:], AF.Abs)
            nc.vector.tensor_scalar(absS[:], absS[:], 1e-20, None, OP.max)
            rca = cp.tile([32, BS], f32, name="rca")
            nc.vector.reciprocal_approx_fast(rca[:], absS[:])
            rat = cp.tile([32, BS], f32, name="rat")
            nc.vector.tensor_tensor(rat[:], b_T[:], rca[:], OP.mult)
            rat2 = cp.tile([32, BS], f32, name="rat2")
            nc.vector.tensor_tensor(rat2[:], rat[:], sgn[:], OP.mult)
            rat_d = dp.tile([32, BS], f32, name="rat_d")
            nc.gpsimd.dma_start(out=rat_d[:], in_=rat2[:])

            if not cfg["lrelu"]:
                # u = 0.505*scale, w = 0.495*|scale| as [32,32] -> [128,16]
                uT = cp.tile([32, BS], f32, name="uT")
                nc.scalar.activation(uT[:], sc_T[:], AF.Copy, scale=0.505)
                wT = cp.tile([32, BS], f32, name="wT")
                nc.scalar.activation(wT[:], sc_T[:], AF.Abs)
                nc.vector.tensor_scalar(wT[:], wT[:], 0.495, None, OP.mult)
                u_d = dp.tile([32, BS], f32, name="u_d")
                nc.gpsimd.dma_start(out=u_d[:], in_=uT[:])
                w_d = dp.tile([32, BS], f32, name="w_d")
                nc.gpsimd.dma_start(out=w_d[:], in_=wT[:])
                uwv = cp.tile([128, 16], f32, name="uwv")
                for i, td in enumerate((u_d, w_d)):
                    for l in range(1, 5):
                        for g in range(2):
                            j = (l - 1) * 2 + g
                            src = AP(tensor=td[:].tensor,
                                     offset=256 * (l - 1) + 16 * g,
                                     ap=((1, 16), (32, 8)))
                            nc.gpsimd.dma_start(out=uwv[:, 8 * i + j:8 * i + j + 1],
                                              in_=src)

            # scatter ratio into zeroed diag blocks scrD[j=(l-1)*2+sg][16,128]
            scrD = dp.tile([8, 16, 128], f32, name="scrD")
            zt = cp.tile([16, 1024], f32, name="zt")
            nc.vector.memset(zt[:], 0.0)
            nc.gpsimd.dma_start(
                out=AP(tensor=scrD[:].tensor, offset=0, ap=((1, 8 * 16 * 128),)),
                in_=zt[:])
            for l in range(1, 5):
                for g in range(2):
                    j = (l - 1) * 2 + g
                    src = AP(tensor=rat_d[:].tensor,
                             offset=256 * (l - 1) + 16 * g,
                             ap=((1, 16), (32, 8)))
                    dst = AP(tensor=scrD[:].tensor, offset=j * 2048,
                             ap=((136, 16), (1, 8)))
                    nc.gpsimd.dma_start(out=dst, in_=src)

            # banded Z'-rank1 stationaries bsb[:, 128 v:...] v = j*4+tau
            bsb = cp.tile([64, 4096], f32, name="bsb")
            nc.vector.memset(bsb[:], 0.0)
            for l in range(1, 5):
                for g in range(2):
                    j = (l - 1) * 2 + g
                    for tau in range(4):
                        v = j * 4 + tau
                        nc.gpsimd.dma_start(
                            out=bsb[16 * tau:16 * tau + 16, 128 * v:128 * (v + 1)],
                            in_=scrD[j])

            # ---------------- main loop
            with tc.tile_pool(name="pd", bufs=5, space="PSUM") as pdp, \
                 tc.tile_pool(name="pv", bufs=2, space="PSUM") as pvp, \
                 tc.tile_pool(name="p5", bufs=1, space="PSUM") as p5p, \
                 tc.tile_pool(name="xin", bufs=2) as xp, \
                 tc.tile_pool(name="dsqp", bufs=2) as dqp, \
                 tc.tile_pool(name="sigp", bufs=3) as sgp2, \
                 tc.tile_pool(name="actp", bufs=2) as app, \
                 tc.tile_pool(name="outp", bufs=2) as opp:

                for g in range(2):
                    for k in range(4):
                        xt = xp.tile([16, 2048], f32, name="xt", tag="xt")
                        nc.sync.dma_start(
                            out=xt[:],
                            in_=x_d[16 * g:16 * g + 16, 2048 * k:2048 * (k + 1)])
                        a_prev = None
                        sig_prev = None
                        for l in range(1, 5):
                            j = (l - 1) * 2 + g
                            dts = []
                            for tau in range(4):
                                dt_ = pdp.tile([128, 512], f32,
                                               name=f"dt{l}{tau}", tag="dt")
                                sl = slice(512 * tau, 512 * (tau + 1))
                                if l == 1:
                                    nc.tensor.matmul(dt_[:], cw["w1b"][:],
                                                     xt[:, sl],
                                                     start=True, stop=False)
                                    nc.tensor.matmul(dt_[:], cw["b1row"][:],
                                                     ones_s[:],
                                                     start=False, stop=True)
                                else:
                                    v2 = (l - 2) * 4 + tau
                                    nc.tensor.matmul(dt_[:], cw[f"wb{l}"][:],
                                                     a_prev[:, sl],
                                                     start=True, stop=False)
                                    nc.tensor.matmul(
                                        dt_[:],
                                        cw["blamb"][:, 128 * v2:128 * (v2 + 1)],
                                        sig_prev[:],
                                        start=False, stop=True)
                                dts.append(dt_)
                            dsq = dqp.tile([128, 2048], f32,
                                           name=f"dsq{l}", tag="dsq")
                            for tau in range(4):
                                sl = slice(512 * tau, 512 * (tau + 1))
                                if cfg["dsq_dve"]:
                                    nc.vector.tensor_tensor(
                                        dsq[:, sl], dts[tau][:], dts[tau][:],
                                        OP.mult)
                                else:
                                    nc.scalar.activation(dsq[:, sl], dts[tau][:],
                                                         AF.Square)
                            vp = pvp.tile([64, 512], f32, name=f"vp{l}", tag="vp")
                            for tau in range(4):
                                sl = slice(512 * tau, 512 * (tau + 1))
                                nc.tensor.matmul(
                                    vp[:], cw["gath"][:, 64 * tau:64 * (tau + 1)],
                                    dsq[:, sl],
                                    start=(tau == 0), stop=(tau == 3))
                            lnv = sgp2.tile([64, 512], f32,
                                            name=f"lnv{l}", tag="lnv")
                            nc.scalar.activation(lnv[:], vp[:], AF.Ln,
                                                 bias=eps_s[:])
                            sig = sgp2.tile([64, 512], f32,
                                            name=f"sig{l}", tag="sig")
                            nc.scalar.activation(sig[:], lnv[:], AF.Exp, scale=0.5)
                            for tau in range(4):
                                v = j * 4 + tau
                                nc.tensor.matmul(
                                    dts[tau][:], bsb[:, 128 * v:128 * (v + 1)],
                                    sig[:],
                                    start=False, stop=True,
                                    skip_group_check=True)
                            anew = app.tile([128, 2048], f32,
                                            name=f"a{l}", tag="a")
                            for tau in range(4):
                                sl = slice(512 * tau, 512 * (tau + 1))
                                if cfg["lrelu"]:
                                    nc.scalar.activation(
                                        anew[:, sl], dts[tau][:], AF.Lrelu,
                                        scale=scv[:, j:j + 1], alpha=0.01)
                                else:
                                    up = app.tile([128, 512], f32,
                                                  name=f"u{l}{tau}", tag="u")
                                    nc.scalar.activation(
                                        up[:], dts[tau][:], AF.Copy,
                                        scale=uwv[:, j:j + 1])
                                    wp_ = app.tile([128, 512], f32,
                                                   name=f"w{l}{tau}", tag="w")
                                    nc.scalar.activation(wp_[:], dts[tau][:],
                                                         AF.Abs)
                                    nc.vector.tensor_scalar(
                                        wp_[:], wp_[:],
                                        uwv[:, 8 + j:9 + j], None, OP.mult)
                                    nc.vector.tensor_tensor(
                                        anew[:, sl], up[:], wp_[:], OP.add)
                            a_prev = anew
                            sig_prev = sig

                        # ---- L5 + un-deferral
                        h5 = p5p.tile([64, 512], f32, name="h5", tag="h5")
                        for tau in range(4):
                            sl = slice(512 * tau, 512 * (tau + 1))
                            nc.tensor.matmul(
                                h5[:], cw["w5b"][:, 64 * tau:64 * (tau + 1)],
                                a_prev[:, sl],
                                start=(tau == 0), stop=False)
                        nc.tensor.matmul(h5[:], cw["b5i"][:], sig_prev[:],
                                         start=False, stop=True)
                        lr5 = opp.tile([64, 512], f32, name="lr5", tag="lr5")
                        if cfg["lrelu"]:
                            nc.scalar.activation(lr5[:], h5[:], AF.Lrelu,
                                                 alpha=0.01)
                        else:
                            sk5 = opp.tile([64, 512], f32, name="sk5", tag="sk5")
                            nc.scalar.activation(sk5[:], h5[:], AF.Copy,
                                                 scale=0.01)
                            nc.vector.tensor_tensor(lr5[:], h5[:], sk5[:], OP.max)
                        rho = opp.tile([64, 512], f32, name="rho", tag="rho")
                        nc.vector.reciprocal_approx_fast(rho[:], sig_prev[:])
                        oc = opp.tile([64, 512], f32, name="oc", tag="oc")
                        nc.vector.tensor_tensor(oc[:], lr5[:], rho[:], OP.mult)
                        ydst = y_d.rearrange(
                            "(sg sp) (kk tau n) -> sg kk tau sp n",
                            sg=2, kk=4, tau=4, n=512)[g, k]
                        # oc partition-major order (p = 16 tau + sp) matches
                        # the (tau, sp, n) iteration of ydst
                        nc.sync.dma_start(out=ydst, in_=oc[:])

    nc.compile()
    return nc


# ------------------------------------------------------------------- runner

def _get_program():
    key = (CFG["dsq_dve"], CFG["lrelu"])
    if key not in _CACHE:
        _CACHE[key] = build_program(CFG)
    return _CACHE[key]


def _make_in_maps(inputs):
    x = np.ascontiguousarray(
        np.asarray(inputs["x"], dtype=np.float32).reshape(B_FULL, T))
    md = np.ascontiguousarray(np.asarray(inputs["metadata"], dtype=np.float32))
    wts = _host_weights(inputs)
    in_maps = []
    for i in range(NCORES):
        m = dict(wts)
        m["x"] = np.ascontiguousarray(x[BS * i:BS * (i + 1)])
        m["metadata"] = np.ascontiguousarray(md[BS * i:BS * (i + 1)])
        in_maps.append(m)
    return in_maps


def run_spmd(inputs, trace=False):
    """Run on all 8 cores; returns (y_full, BassKernelResults)."""
    from concourse.bass_utils import run_bass_kernel_spmd
    nc = _get_program()
    in_maps = _make_in_maps(inputs)
    res = run_bass_kernel_spmd(nc, in_maps, core_ids=list(range(NCORES)),
                               trace=trace)
    y = np.concatenate([np.asarray(r["y"]) for r in res.results], axis=0)
    y = y.reshape(B_FULL, 1, T).astype(np.float32)
    return y, res


def kernel(**inputs):
    y, _ = run_spmd(inputs, trace=False)
    return y


# revision 19
# speedup vs baseline: 2.0508x; 2.0508x over previous
"""Trainium2 Bass kernel for nn_AttenuationToRainRate (dense_mlp).

Architecture notes
------------------
Reference computation per (sample b, position t):
  style MLP: metadata (16) -> 64 -> 128 -> 64, split into 4 x (scale, bias)[8]
  main chain: x -> [w1] -> adain/lrelu -> [w2] -> adain/lrelu -> [w3] ->
              adain/lrelu -> [w4] -> adain/lrelu -> [w5] -> lrelu
  adain(h) = scale * (h - mean_c h) / (std_ddof1 + 1e-6) + bias

Key algebraic transform (deferred normalization): lrelu is positively
homogeneous, and the channel-normalization d/sigma is invariant to any
positive per-position scaling of d.  We therefore never divide by sigma
inside the chain; instead we track activations scaled by lambda = sigma~
(the unnormalized std) and fold the division into:
  - per-layer bias terms, realized as PE rank-1 accumulates (b' (x) lambda)
  - the adain affine, realized as Z' = d~ + (bias/scale) (x) sigma~ followed
    by one ACT pass: a~ = Lrelu(scale * Z') (= sigma~ * lrelu(z), exact)
  - a single reciprocal at the very end: out = lrelu(h5~) * (1/sigma4~)
Mean-removal is folded into the weights host-side: W' = W (I - J/8).

Data layout per core (32 samples, data-parallel over 8 cores):
  tile [128, 512]: partition p = 8*s' + c (16 samples x 8 channels),
  free = 512 consecutive positions.  Per supergroup sg (16 samples) and
  cohort k, four tiles tau cover 2048 positions.  sigma~^2 for the 4 tiles
  of a cohort is gathered (via banded PE stationaries) into one packed
  [64, 512] psum tile (partition q = 16*tau + s'), so the ln/exp sigma chain
  runs once per cohort instead of once per tile.
"""

import numpy as np

B_FULL, T = 256, 8192
NCORES = 8
BS = B_FULL // NCORES  # 32 samples per core
F = 16

# config switches (test.py may flip these and call _reset())
CFG = {
    "mm_r": True,     # bitcast matmul operands to float32r (1 cyc/row at N>=256)
    "prelu": True,    # Prelu (in the ln/exp act-table set) instead of Lrelu
    "rho_exp": True,  # rho4 = Exp(-0.5 ln v4) on ACT instead of DVE reciprocal
}

_CACHE = {}


def _reset():
    _CACHE.clear()


# ----------------------------------------------------------------- host side

def _host_weights(inp):
    """Build all weight-derived constant tensors (f32 numpy, device layouts)."""
    f64 = np.float64
    I8 = np.eye(8, dtype=f64)
    C = I8 - np.full((8, 8), 1.0 / 8.0, dtype=f64)  # output-centering

    w = {}
    w1 = np.asarray(inp["w1"], dtype=f64)           # (1, 8)
    b1 = np.asarray(inp["b1"], dtype=f64)           # (8,)
    w1p = (w1 @ C)[0]
    b1p = b1 - b1.mean()

    w1b = np.zeros((16, 128), dtype=f64)
    b1row = np.zeros((1, 128), dtype=f64)
    for s in range(16):
        w1b[s, 8 * s:8 * s + 8] = w1p
        b1row[0, 8 * s:8 * s + 8] = b1p
    w["w1b"] = w1b
    w["b1row"] = b1row

    blamb = np.zeros((64, 12 * 128), dtype=f64)
    for l in (2, 3, 4):
        W = np.asarray(inp[f"w{l}"], dtype=f64) @ C   # (8, 8) in->out
        bp = np.asarray(inp[f"b{l}"], dtype=f64)
        bp = bp - bp.mean()
        wb = np.zeros((128, 128), dtype=f64)
        for s in range(16):
            wb[8 * s:8 * s + 8, 8 * s:8 * s + 8] = W
        w[f"wb{l}"] = wb
        for tau in range(4):
            v = (l - 2) * 4 + tau
            blk = blamb[:, 128 * v:128 * (v + 1)]
            for s in range(16):
                blk[16 * tau + s, 8 * s:8 * s + 8] = bp
    w["blamb"] = blamb

    gath = np.zeros((128, 4 * 64), dtype=f64)
    w5b = np.zeros((128, 4 * 64), dtype=f64)
    w5 = np.asarray(inp["w5"], dtype=f64)[:, 0]      # (8,)
    for tau in range(4):
        for s in range(16):
            for c in range(8):
                gath[8 * s + c, 64 * tau + 16 * tau + s] = 1.0 / 7.0
                w5b[8 * s + c, 64 * tau + 16 * tau + s] = w5[c]
    w["gath"] = gath
    w["w5b"] = w5b
    w["b5i"] = np.eye(64, dtype=f64) * float(np.asarray(inp["b5"], dtype=f64)[0])

    w["mw1"] = np.asarray(inp["mw1"], dtype=f64)
    w["mw2"] = np.asarray(inp["mw2"], dtype=f64)
    w["mw3"] = np.asarray(inp["mw3"], dtype=f64)
    w["mb1c"] = np.asarray(inp["mb1"], dtype=f64).reshape(64, 1)
    w["mb2c"] = np.asarray(inp["mb2"], dtype=f64).reshape(128, 1)
    w["mb3c"] = np.asarray(inp["mb3"], dtype=f64).reshape(64, 1)

    return {k: np.ascontiguousarray(v, dtype=np.float32) for k, v in w.items()}


_WSHAPES = {
    "w1b": [16, 128], "b1row": [1, 128],
    "wb2": [128, 128], "wb3": [128, 128], "wb4": [128, 128],
    "blamb": [64, 1536], "gath": [128, 256], "w5b": [128, 256], "b5i": [64, 64],
    "mw1": [16, 64], "mw2": [64, 128], "mw3": [128, 64],
    "mb1c": [64, 1], "mb2c": [128, 1], "mb3c": [64, 1],
}


# --------------------------------------------------------------- device side

def build_program(cfg=None):
    import concourse.bacc as bacc
    import concourse.mybir as mybir
    from concourse.ap import AP
    from concourse.tile import TileContext

    cfg = dict(CFG if cfg is None else cfg)
    f32 = mybir.dt.float32
    mdt = mybir.dt.float32r if cfg["mm_r"] else f32
    AF = mybir.ActivationFunctionType
    OP = mybir.AluOpType

    nc = bacc.Bacc("TRN2", target_bir_lowering=False)
    x_d = nc.dram_tensor("x", [BS, T], f32, kind="ExternalInput")
    md_d = nc.dram_tensor("metadata", [BS, F], f32, kind="ExternalInput")
    y_d = nc.dram_tensor("y", [BS, T], f32, kind="ExternalOutput")
    _MM_STAT = {"w1b", "b1row", "wb2", "wb3", "wb4", "blamb", "gath",
                "w5b", "b5i"}
    wd = {name: nc.dram_tensor(name, shp, mdt if name in _MM_STAT else f32,
                               kind="ExternalInput")
          for name, shp in _WSHAPES.items()}

    with TileContext(nc) as tc:
        with tc.tile_pool(name="const", bufs=1) as cp, \
             tc.tile_pool(name="scr", bufs=1, space="DRAM") as dp:

            # ---- constants to SBUF
            cw = {}
            for name, shp in _WSHAPES.items():
                t = cp.tile(shp, mdt if name in _MM_STAT else f32,
                            name=f"c_{name}")
                nc.sync.dma_start(out=t[:], in_=wd[name][:])
                cw[name] = t
            ones_s = cp.tile([1, 512], mdt, name="ones_s")
            nc.vector.memset(ones_s[:].bitcast(f32), 1.0)
            eps_s = cp.tile([64, 1], f32, name="eps_s")
            nc.vector.memset(eps_s[:], 1e-12)

            # ---- style MLP (per-core 32 samples)
            with tc.tile_pool(name="stp", bufs=1, space="PSUM") as sp:
                mdT = cp.tile([F, BS], f32, name="mdT")
                nc.sync.dma_start(out=mdT[:], in_=md_d.rearrange("s f -> f s"))
                ps1 = sp.tile([64, BS], f32, name="ps1")
                nc.tensor.matmul(ps1[:], cw["mw1"][:], mdT[:], start=True, stop=True)
                s1 = cp.tile([64, BS], f32, name="s1")
                nc.scalar.activation(s1[:], ps1[:], AF.Relu, bias=cw["mb1c"][:])
                ps2 = sp.tile([128, BS], f32, name="ps2")
                nc.tensor.matmul(ps2[:], cw["mw2"][:], s1[:], start=True, stop=True)
                s2 = cp.tile([128, BS], f32, name="s2")
                nc.scalar.activation(s2[:], ps2[:], AF.Relu, bias=cw["mb2c"][:])
                ps3 = sp.tile([64, BS], f32, name="ps3")
                nc.tensor.matmul(ps3[:], cw["mw3"][:], s2[:], start=True, stop=True)
                sT = cp.tile([64, BS], f32, name="sT")
                nc.scalar.activation(sT[:], ps3[:], AF.Identity, bias=cw["mb3c"][:])

            # ---- style-derived constants via DRAM round-trips
            # sT[16(l-1)+2c + (0:scale,1:bias), 16 sg + s']
            sT_d = dp.tile([64, BS], f32, name="sT_d")
            nc.gpsimd.dma_start(out=sT_d[:], in_=sT[:])

            # scale vectors [128, 8]: col j = (l-1)*2+sg ; partition 8 s' + c
            scv = cp.tile([128, 8], f32, name="scv")
            for l in range(1, 5):
                for g in range(2):
                    j = (l - 1) * 2 + g
                    src = AP(tensor=sT_d[:].tensor,
                             offset=512 * (l - 1) + 16 * g,
                             ap=((1, 16), (64, 8)))
                    nc.gpsimd.dma_start(out=scv[:, j:j + 1], in_=src)

            # scale/bias as [32, 32] (row q = 8(l-1)+c)
            sc_T = cp.tile([32, BS], f32, name="sc_T")
            nc.gpsimd.dma_start(
                out=sc_T[:],
                in_=sT_d[:].rearrange("(m two) s -> two m s", two=2)[0])
            b_T = cp.tile([32, BS], f32, name="b_T")
            nc.gpsimd.dma_start(
                out=b_T[:],
                in_=sT_d[:].rearrange("(m two) s -> two m s", two=2)[1])

            # ratio = bias/scale with sign-preserving clamp of |scale|>=1e-20
            sgn = cp.tile([32, BS], f32, name="sgn")
            nc.scalar.sign(sgn[:], sc_T[:])
            absS = cp.tile([32, BS], f32, name="absS")
            nc.scalar.activation(absS[:], sc_T[:], AF.Abs)
            nc.vector.tensor_scalar(absS[:], absS[:], 1e-20, None, OP.max)
            rca = cp.tile([32, BS], f32, name="rca")
            nc.vector.reciprocal_approx_fast(rca[:], absS[:])
            rat = cp.tile([32, BS], f32, name="rat")
            nc.vector.tensor_tensor(rat[:], b_T[:], rca[:], OP.mult)
            rat2 = cp.tile([32, BS], f32, name="rat2")
            nc.vector.tensor_tensor(rat2[:], rat[:], sgn[:], OP.mult)
            rat_d = dp.tile([32, BS], f32, name="rat_d")
            nc.gpsimd.dma_start(out=rat_d[:], in_=rat2[:])

            # scatter ratio into zeroed diag blocks scrD[j=(l-1)*2+sg][16,128]
            scrD = dp.tile([8, 16, 128], f32, name="scrD")
            zt = cp.tile([16, 1024], f32, name="zt")
            nc.vector.memset(zt[:], 0.0)
            nc.gpsimd.dma_start(
                out=AP(tensor=scrD[:].tensor, offset=0, ap=((1, 8 * 16 * 128),)),
                in_=zt[:])
            for l in range(1, 5):
                for g in range(2):
                    j = (l - 1) * 2 + g
                    src = AP(tensor=rat_d[:].tensor,
                             offset=256 * (l - 1) + 16 * g,
                             ap=((1, 16), (32, 8)))
                    dst = AP(tensor=scrD[:].tensor, offset=j * 2048,
                             ap=((136, 16), (1, 8)))
                    nc.gpsimd.dma_start(out=dst, in_=src)

            # banded Z'-rank1 stationaries bsb[:, 128 v:...] v = j*4+tau
            bsb = cp.tile([64, 4096], mdt, name="bsb")
            nc.vector.memset(bsb[:].bitcast(f32), 0.0)
            for l in range(1, 5):
                for g in range(2):
                    j = (l - 1) * 2 + g
                    for tau in range(4):
                        v = j * 4 + tau
                        nc.gpsimd.dma_start(
                            out=bsb[16 * tau:16 * tau + 16, 128 * v:128 * (v + 1)],
                            in_=scrD[j].bitcast(mdt))

            # ---------------- main loop
            AF_LREL = AF.Prelu if cfg["prelu"] else AF.Lrelu

            with tc.tile_pool(name="pd", bufs=3, space="PSUM") as pdp, \
                 tc.tile_pool(name="pv", bufs=1, space="PSUM") as pvp, \
                 tc.tile_pool(name="p5", bufs=1, space="PSUM") as p5p, \
                 tc.tile_pool(name="xin", bufs=2) as xp, \
                 tc.tile_pool(name="dsqp", bufs=2) as dqp, \
                 tc.tile_pool(name="sigp", bufs=3) as sgp2, \
                 tc.tile_pool(name="actp", bufs=2) as app, \
                 tc.tile_pool(name="outp", bufs=2) as opp:

                for g in range(2):
                    for k in range(4):
                        xt = xp.tile([16, 2048], mdt, name="xt", tag="xt")
                        nc.sync.dma_start(
                            out=xt[:],
                            in_=x_d[16 * g:16 * g + 16,
                                    2048 * k:2048 * (k + 1)].bitcast(mdt))
                        a_prev = None
                        sig_prev = None
                        lnv_last = None
                        for l in range(1, 5):
                            j = (l - 1) * 2 + g
                            # two [128,1024] psum pair-tiles; tau -> half
                            prs = [pdp.tile([128, 1024], f32,
                                            name=f"dt{l}{p}", tag="dt")
                                   for p in range(2)]

                            def dtap(tau):
                                h = tau % 2
                                return prs[tau // 2][:, 512 * h:512 * (h + 1)]

                            for tau in range(4):
                                sl = slice(512 * tau, 512 * (tau + 1))
                                if l == 1:
                                    nc.tensor.matmul(dtap(tau), cw["w1b"][:],
                                                     xt[:, sl],
                                                     start=True, stop=False)
                                    nc.tensor.matmul(dtap(tau),
                                                     cw["b1row"][:],
                                                     ones_s[:],
                                                     start=False, stop=True)
                                else:
                                    v2 = (l - 2) * 4 + tau
                                    nc.tensor.matmul(dtap(tau),
                                                     cw[f"wb{l}"][:],
                                                     a_prev[:, sl],
                                                     start=True, stop=False)
                                    nc.tensor.matmul(
                                        dtap(tau),
                                        cw["blamb"][:, 128 * v2:128 * (v2 + 1)],
                                        sig_prev[:],
                                        start=False, stop=True)
                            dsq = dqp.tile([128, 2048], mdt,
                                           name=f"dsq{l}", tag="dsq")
                            for p in range(2):
                                nc.scalar.activation(
                                    dsq[:, 1024 * p:1024 * (p + 1)],
                                    prs[p][:], AF.Square)
                            vp = pvp.tile([64, 512], f32, name=f"vp{l}", tag="vp")
                            for tau in range(4):
                                sl = slice(512 * tau, 512 * (tau + 1))
                                nc.tensor.matmul(
                                    vp[:],
                                    cw["gath"][:, 64 * tau:64 * (tau + 1)],
                                    dsq[:, sl],
                                    start=(tau == 0), stop=(tau == 3))
                            lnv = sgp2.tile([64, 512], f32,
                                            name=f"lnv{l}", tag="lnv")
                            nc.scalar.activation(lnv[:], vp[:], AF.Ln,
                                                 bias=eps_s[:])
                            sig = sgp2.tile([64, 512], mdt,
                                            name=f"sig{l}", tag="sig")
                            nc.scalar.activation(sig[:], lnv[:], AF.Exp, scale=0.5)
                            for tau in range(4):
                                v = j * 4 + tau
                                nc.tensor.matmul(
                                    dtap(tau),
                                    bsb[:, 128 * v:128 * (v + 1)],
                                    sig[:],
                                    start=False, stop=True,
                                    skip_group_check=True)
                            anew = app.tile([128, 2048], mdt,
                                            name=f"a{l}", tag="a")
                            for p in range(2):
                                nc.scalar.activation(
                                    anew[:, 1024 * p:1024 * (p + 1)],
                                    prs[p][:], AF_LREL,
                                    scale=scv[:, j:j + 1], alpha=0.01)
                            a_prev = anew
                            sig_prev = sig
                            lnv_last = lnv

                        # ---- L5 + un-deferral
                        h5 = p5p.tile([64, 512], f32, name="h5", tag="h5")
                        for tau in range(4):
                            sl = slice(512 * tau, 512 * (tau + 1))
                            nc.tensor.matmul(
                                h5[:],
                                cw["w5b"][:, 64 * tau:64 * (tau + 1)],
                                a_prev[:, sl],
                                start=(tau == 0), stop=False)
                        nc.tensor.matmul(h5[:], cw["b5i"][:], sig_prev[:],
                                         start=False, stop=True)
                        lr5 = opp.tile([64, 512], f32, name="lr5", tag="lr5")
                        nc.scalar.activation(lr5[:], h5[:], AF_LREL, alpha=0.01)
                        rho = opp.tile([64, 512], f32, name="rho", tag="rho")
                        if cfg["rho_exp"]:
                            nc.scalar.activation(rho[:], lnv_last[:], AF.Exp,
                                                 scale=-0.5)
                        else:
                            nc.vector.reciprocal_approx_fast(rho[:], sig_prev[:])
                        oc = opp.tile([64, 512], f32, name="oc", tag="oc")
                        nc.vector.tensor_tensor(oc[:], lr5[:], rho[:], OP.mult)
                        ydst = y_d.rearrange(
                            "(sg sp) (kk tau n) -> sg kk tau sp n",
                            sg=2, kk=4, tau=4, n=512)[g, k]
                        # oc partition-major order (p = 16 tau + sp) matches
                        # the (tau, sp, n) iteration of ydst
                        nc.sync.dma_start(out=ydst, in_=oc[:])

    nc.compile()
    return nc


# ------------------------------------------------------------------- runner

def _get_program():
    key = tuple(sorted(CFG.items()))
    if key not in _CACHE:
        _CACHE[key] = build_program(CFG)
    return _CACHE[key]


def _make_in_maps(inputs):
    x = np.ascontiguousarray(
        np.asarray(inputs["x"], dtype=np.float32).reshape(B_FULL, T))
    md = np.ascontiguousarray(np.asarray(inputs["metadata"], dtype=np.float32))
    wts = _host_weights(inputs)
    in_maps = []
    for i in range(NCORES):
        m = dict(wts)
        m["x"] = np.ascontiguousarray(x[BS * i:BS * (i + 1)])
        m["metadata"] = np.ascontiguousarray(md[BS * i:BS * (i + 1)])
        in_maps.append(m)
    return in_maps


def run_spmd(inputs, trace=False):
    """Run on all 8 cores; returns (y_full, BassKernelResults)."""
    from concourse.bass_utils import run_bass_kernel_spmd
    nc = _get_program()
    in_maps = _make_in_maps(inputs)
    res = run_bass_kernel_spmd(nc, in_maps, core_ids=list(range(NCORES)),
                               trace=trace)
    y = np.concatenate([np.asarray(r["y"]) for r in res.results], axis=0)
    y = y.reshape(B_FULL, 1, T).astype(np.float32)
    return y, res


def kernel(**inputs):
    y, _ = run_spmd(inputs, trace=False)
    return y


# revision 20
# speedup vs baseline: 2.2657x; 1.1048x over previous
"""Trainium2 Bass kernel for nn_AttenuationToRainRate (dense_mlp).

Architecture notes
------------------
Reference computation per (sample b, position t):
  style MLP: metadata (16) -> 64 -> 128 -> 64, split into 4 x (scale, bias)[8]
  main chain: x -> [w1] -> adain/lrelu -> [w2] -> adain/lrelu -> [w3] ->
              adain/lrelu -> [w4] -> adain/lrelu -> [w5] -> lrelu
  adain(h) = scale * (h - mean_c h) / (std_ddof1 + 1e-6) + bias

Key algebraic transform (deferred normalization): lrelu is positively
homogeneous, and the channel-normalization d/sigma is invariant to any
positive per-position scaling of d.  We therefore never divide by sigma
inside the chain; instead we track activations scaled by lambda = sigma~
(the unnormalized std) and fold the division into:
  - per-layer bias terms, realized as PE rank-1 accumulates (b' (x) lambda)
  - the adain affine, realized as Z' = d~ + (bias/scale) (x) sigma~ followed
    by one ACT pass: a~ = Lrelu(scale * Z') (= sigma~ * lrelu(z), exact)
  - a single reciprocal at the very end: out = lrelu(h5~) * (1/sigma4~)
Mean-removal is folded into the weights host-side: W' = W (I - J/8).

Data layout per core (32 samples, data-parallel over 8 cores):
  tile [128, 512]: partition p = 8*s' + c (16 samples x 8 channels),
  free = 512 consecutive positions.  Per supergroup sg (16 samples) and
  cohort k, four tiles tau cover 2048 positions.  sigma~^2 for the 4 tiles
  of a cohort is gathered (via banded PE stationaries) into one packed
  [64, 512] psum tile (partition q = 16*tau + s'), so the ln/exp sigma chain
  runs once per cohort instead of once per tile.
"""

import numpy as np

B_FULL, T = 256, 8192
NCORES = 8
BS = B_FULL // NCORES  # 32 samples per core
F = 16

# config switches (test.py may flip these and call _reset())
CFG = {
    "mm_r": True,     # bitcast matmul operands to float32r (1 cyc/row at N>=256)
    "prelu": True,    # Prelu (in the ln/exp act-table set) instead of Lrelu
    "rho_exp": True,  # rho4 = Exp(-0.5 ln v4) on ACT instead of DVE reciprocal
}

_CACHE = {}


def _reset():
    _CACHE.clear()


# ----------------------------------------------------------------- host side

def _host_weights(inp):
    """Build all weight-derived constant tensors (f32 numpy, device layouts)."""
    f64 = np.float64
    I8 = np.eye(8, dtype=f64)
    C = I8 - np.full((8, 8), 1.0 / 8.0, dtype=f64)  # output-centering

    w = {}
    w1 = np.asarray(inp["w1"], dtype=f64)           # (1, 8)
    b1 = np.asarray(inp["b1"], dtype=f64)           # (8,)
    w1p = (w1 @ C)[0]
    b1p = b1 - b1.mean()

    w1b = np.zeros((16, 128), dtype=f64)
    b1row = np.zeros((1, 128), dtype=f64)
    for s in range(16):
        w1b[s, 8 * s:8 * s + 8] = w1p
        b1row[0, 8 * s:8 * s + 8] = b1p
    w["w1b"] = w1b
    w["b1row"] = b1row

    blamb = np.zeros((64, 12 * 128), dtype=f64)
    for l in (2, 3, 4):
        W = np.asarray(inp[f"w{l}"], dtype=f64) @ C   # (8, 8) in->out
        bp = np.asarray(inp[f"b{l}"], dtype=f64)
        bp = bp - bp.mean()
        wb = np.zeros((128, 128), dtype=f64)
        for s in range(16):
            wb[8 * s:8 * s + 8, 8 * s:8 * s + 8] = W
        w[f"wb{l}"] = wb
        for tau in range(4):
            v = (l - 2) * 4 + tau
            blk = blamb[:, 128 * v:128 * (v + 1)]
            for s in range(16):
                blk[16 * tau + s, 8 * s:8 * s + 8] = bp
    w["blamb"] = blamb

    gath = np.zeros((128, 4 * 64), dtype=f64)
    w5b = np.zeros((128, 4 * 64), dtype=f64)
    w5 = np.asarray(inp["w5"], dtype=f64)[:, 0]      # (8,)
    for tau in range(4):
        for s in range(16):
            for c in range(8):
                gath[8 * s + c, 64 * tau + 16 * tau + s] = 1.0 / 7.0
                w5b[8 * s + c, 64 * tau + 16 * tau + s] = w5[c]
    w["gath"] = gath
    w["w5b"] = w5b
    w["b5i"] = np.eye(64, dtype=f64) * float(np.asarray(inp["b5"], dtype=f64)[0])

    w["mw1"] = np.asarray(inp["mw1"], dtype=f64)
    w["mw2"] = np.asarray(inp["mw2"], dtype=f64)
    w["mw3"] = np.asarray(inp["mw3"], dtype=f64)
    w["mb1c"] = np.asarray(inp["mb1"], dtype=f64).reshape(64, 1)
    w["mb2c"] = np.asarray(inp["mb2"], dtype=f64).reshape(128, 1)
    w["mb3c"] = np.asarray(inp["mb3"], dtype=f64).reshape(64, 1)

    return {k: np.ascontiguousarray(v, dtype=np.float32) for k, v in w.items()}


_WSHAPES = {
    "w1b": [16, 128], "b1row": [1, 128],
    "wb2": [128, 128], "wb3": [128, 128], "wb4": [128, 128],
    "blamb": [64, 1536], "gath": [128, 256], "w5b": [128, 256], "b5i": [64, 64],
    "mw1": [16, 64], "mw2": [64, 128], "mw3": [128, 64],
    "mb1c": [64, 1], "mb2c": [128, 1], "mb3c": [64, 1],
}


# --------------------------------------------------------------- device side

def build_program(cfg=None):
    import concourse.bacc as bacc
    import concourse.mybir as mybir
    from concourse.ap import AP
    from concourse.tile import TileContext

    cfg = dict(CFG if cfg is None else cfg)
    f32 = mybir.dt.float32
    mdt = mybir.dt.float32r if cfg["mm_r"] else f32
    AF = mybir.ActivationFunctionType
    OP = mybir.AluOpType

    class _KBacc(bacc.Bacc):
        # The stock insert_act_table_loads picks the FIRST table set
        # containing each activation function, which alternates between
        # exp_and_others and natural_log for this kernel's Square/Ln/Exp/
        # Prelu mix -> one ~2.7us ACT_TABLE_LOAD per transition.  Every
        # function we use lives in natural_log_exp_and_others, so blank
        # all other sets (keeping list positions = set ids) to force a
        # single resident table.
        def insert_act_table_loads(self):
            import concourse.mybir as _mb
            from concourse.hw_specs import get_activation_tables
            import concourse._compat as _cc
            has_activation = any(
                isinstance(i, _mb.InstActivation)
                for b in self.main_func.blocks
                for i in b.instructions
            )
            if not has_activation:
                return
            tables = []
            for name, funcs in get_activation_tables(self.m.arch).items():
                keep = funcs if name == "natural_log_exp_and_others" else set()
                tables.append((name, keep))
            bacc._bass_rust.insert_act_table_loads(self, tables)

    nc = _KBacc("TRN2", target_bir_lowering=False)
    x_d = nc.dram_tensor("x", [BS, T], f32, kind="ExternalInput")
    md_d = nc.dram_tensor("metadata", [BS, F], f32, kind="ExternalInput")
    y_d = nc.dram_tensor("y", [BS, T], f32, kind="ExternalOutput")
    _MM_STAT = {"w1b", "b1row", "wb2", "wb3", "wb4", "blamb", "gath",
                "w5b", "b5i"}
    wd = {name: nc.dram_tensor(name, shp, mdt if name in _MM_STAT else f32,
                               kind="ExternalInput")
          for name, shp in _WSHAPES.items()}

    with TileContext(nc) as tc:
        with tc.tile_pool(name="const", bufs=1) as cp, \
             tc.tile_pool(name="scr", bufs=1, space="DRAM") as dp:

            # ---- constants to SBUF
            cw = {}
            for name, shp in _WSHAPES.items():
                t = cp.tile(shp, mdt if name in _MM_STAT else f32,
                            name=f"c_{name}")
                nc.sync.dma_start(out=t[:], in_=wd[name][:])
                cw[name] = t
            ones_s = cp.tile([1, 512], mdt, name="ones_s")
            nc.vector.memset(ones_s[:].bitcast(f32), 1.0)
            eps_s = cp.tile([64, 1], f32, name="eps_s")
            nc.vector.memset(eps_s[:], 1e-12)

            # ---- style MLP (per-core 32 samples)
            with tc.tile_pool(name="stp", bufs=1, space="PSUM") as sp:
                mdT = cp.tile([F, BS], f32, name="mdT")
                nc.sync.dma_start(out=mdT[:], in_=md_d.rearrange("s f -> f s"))
                ps1 = sp.tile([64, BS], f32, name="ps1")
                nc.tensor.matmul(ps1[:], cw["mw1"][:], mdT[:], start=True, stop=True)
                s1 = cp.tile([64, BS], f32, name="s1")
                nc.scalar.activation(s1[:], ps1[:], AF.Relu, bias=cw["mb1c"][:])
                ps2 = sp.tile([128, BS], f32, name="ps2")
                nc.tensor.matmul(ps2[:], cw["mw2"][:], s1[:], start=True, stop=True)
                s2 = cp.tile([128, BS], f32, name="s2")
                nc.scalar.activation(s2[:], ps2[:], AF.Relu, bias=cw["mb2c"][:])
                ps3 = sp.tile([64, BS], f32, name="ps3")
                nc.tensor.matmul(ps3[:], cw["mw3"][:], s2[:], start=True, stop=True)
                sT = cp.tile([64, BS], f32, name="sT")
                nc.scalar.activation(sT[:], ps3[:], AF.Identity, bias=cw["mb3c"][:])

            # ---- style-derived constants via DRAM round-trips
            # sT[16(l-1)+2c + (0:scale,1:bias), 16 sg + s']
            sT_d = dp.tile([64, BS], f32, name="sT_d")
            nc.gpsimd.dma_start(out=sT_d[:], in_=sT[:])

            # scale vectors [128, 8]: col j = (l-1)*2+sg ; partition 8 s' + c
            scv = cp.tile([128, 8], f32, name="scv")
            for l in range(1, 5):
                for g in range(2):
                    j = (l - 1) * 2 + g
                    src = AP(tensor=sT_d[:].tensor,
                             offset=512 * (l - 1) + 16 * g,
                             ap=((1, 16), (64, 8)))
                    nc.gpsimd.dma_start(out=scv[:, j:j + 1], in_=src)

            # scale/bias as [32, 32] (row q = 8(l-1)+c)
            sc_T = cp.tile([32, BS], f32, name="sc_T")
            nc.gpsimd.dma_start(
                out=sc_T[:],
                in_=sT_d[:].rearrange("(m two) s -> two m s", two=2)[0])
            b_T = cp.tile([32, BS], f32, name="b_T")
            nc.gpsimd.dma_start(
                out=b_T[:],
                in_=sT_d[:].rearrange("(m two) s -> two m s", two=2)[1])

            # ratio = bias/scale with sign-preserving clamp of |scale|>=1e-20
            sgn = cp.tile([32, BS], f32, name="sgn")
            nc.scalar.sign(sgn[:], sc_T[:])
            absS = cp.tile([32, BS], f32, name="absS")
            nc.scalar.activation(absS[:], sc_T[:], AF.Abs)
            nc.vector.tensor_scalar(absS[:], absS[:], 1e-20, None, OP.max)
            rca = cp.tile([32, BS], f32, name="rca")
            nc.vector.reciprocal_approx_fast(rca[:], absS[:])
            rat = cp.tile([32, BS], f32, name="rat")
            nc.vector.tensor_tensor(rat[:], b_T[:], rca[:], OP.mult)
            rat2 = cp.tile([32, BS], f32, name="rat2")
            nc.vector.tensor_tensor(rat2[:], rat[:], sgn[:], OP.mult)
            rat_d = dp.tile([32, BS], f32, name="rat_d")
            nc.gpsimd.dma_start(out=rat_d[:], in_=rat2[:])

            # scatter ratio into zeroed diag blocks scrD[j=(l-1)*2+sg][16,128]
            scrD = dp.tile([8, 16, 128], f32, name="scrD")
            zt = cp.tile([16, 1024], f32, name="zt")
            nc.vector.memset(zt[:], 0.0)
            nc.gpsimd.dma_start(
                out=AP(tensor=scrD[:].tensor, offset=0, ap=((1, 8 * 16 * 128),)),
                in_=zt[:])
            for l in range(1, 5):
                for g in range(2):
                    j = (l - 1) * 2 + g
                    src = AP(tensor=rat_d[:].tensor,
                             offset=256 * (l - 1) + 16 * g,
                             ap=((1, 16), (32, 8)))
                    dst = AP(tensor=scrD[:].tensor, offset=j * 2048,
                             ap=((136, 16), (1, 8)))
                    nc.gpsimd.dma_start(out=dst, in_=src)

            # banded Z'-rank1 stationaries bsb[:, 128 v:...] v = j*4+tau
            bsb = cp.tile([64, 4096], mdt, name="bsb")
            nc.vector.memset(bsb[:].bitcast(f32), 0.0)
            for l in range(1, 5):
                for g in range(2):
                    j = (l - 1) * 2 + g
                    for tau in range(4):
                        v = j * 4 + tau
                        nc.gpsimd.dma_start(
                            out=bsb[16 * tau:16 * tau + 16, 128 * v:128 * (v + 1)],
                            in_=scrD[j].bitcast(mdt))

            # ---------------- main loop
            AF_LREL = AF.Prelu if cfg["prelu"] else AF.Lrelu

            with tc.tile_pool(name="pd", bufs=3, space="PSUM") as pdp, \
                 tc.tile_pool(name="pv", bufs=1, space="PSUM") as pvp, \
                 tc.tile_pool(name="p5", bufs=1, space="PSUM") as p5p, \
                 tc.tile_pool(name="xin", bufs=2) as xp, \
                 tc.tile_pool(name="dsqp", bufs=2) as dqp, \
                 tc.tile_pool(name="sigp", bufs=3) as sgp2, \
                 tc.tile_pool(name="actp", bufs=2) as app, \
                 tc.tile_pool(name="outp", bufs=2) as opp:

                for g in range(2):
                    for k in range(4):
                        xt = xp.tile([16, 2048], mdt, name="xt", tag="xt")
                        nc.sync.dma_start(
                            out=xt[:],
                            in_=x_d[16 * g:16 * g + 16,
                                    2048 * k:2048 * (k + 1)].bitcast(mdt))
                        a_prev = None
                        sig_prev = None
                        lnv_last = None
                        for l in range(1, 5):
                            j = (l - 1) * 2 + g
                            # two [128,1024] psum pair-tiles; tau -> half
                            prs = [pdp.tile([128, 1024], f32,
                                            name=f"dt{l}{p}", tag="dt")
                                   for p in range(2)]

                            def dtap(tau):
                                h = tau % 2
                                return prs[tau // 2][:, 512 * h:512 * (h + 1)]

                            for tau in range(4):
                                sl = slice(512 * tau, 512 * (tau + 1))
                                if l == 1:
                                    nc.tensor.matmul(dtap(tau), cw["w1b"][:],
                                                     xt[:, sl],
                                                     start=True, stop=False)
                                    nc.tensor.matmul(dtap(tau),
                                                     cw["b1row"][:],
                                                     ones_s[:],
                                                     start=False, stop=True)
                                else:
                                    v2 = (l - 2) * 4 + tau
                                    nc.tensor.matmul(dtap(tau),
                                                     cw[f"wb{l}"][:],
                                                     a_prev[:, sl],
                                                     start=True, stop=False)
                                    nc.tensor.matmul(
                                        dtap(tau),
                                        cw["blamb"][:, 128 * v2:128 * (v2 + 1)],
                                        sig_prev[:],
                                        start=False, stop=True)
                            dsq = dqp.tile([128, 2048], mdt,
                                           name=f"dsq{l}", tag="dsq")
                            for p in range(2):
                                nc.scalar.activation(
                                    dsq[:, 1024 * p:1024 * (p + 1)],
                                    prs[p][:], AF.Square)
                            vp = pvp.tile([64, 512], f32, name=f"vp{l}", tag="vp")
                            for tau in range(4):
                                sl = slice(512 * tau, 512 * (tau + 1))
                                nc.tensor.matmul(
                                    vp[:],
                                    cw["gath"][:, 64 * tau:64 * (tau + 1)],
                                    dsq[:, sl],
                                    start=(tau == 0), stop=(tau == 3))
                            lnv = sgp2.tile([64, 512], f32,
                                            name=f"lnv{l}", tag="lnv")
                            nc.scalar.activation(lnv[:], vp[:], AF.Ln,
                                                 bias=eps_s[:])
                            sig = sgp2.tile([64, 512], mdt,
                                            name=f"sig{l}", tag="sig")
                            nc.scalar.activation(sig[:], lnv[:], AF.Exp, scale=0.5)
                            for tau in range(4):
                                v = j * 4 + tau
                                nc.tensor.matmul(
                                    dtap(tau),
                                    bsb[:, 128 * v:128 * (v + 1)],
                                    sig[:],
                                    start=False, stop=True,
                                    skip_group_check=True)
                            anew = app.tile([128, 2048], mdt,
                                            name=f"a{l}", tag="a")
                            for p in range(2):
                                nc.scalar.activation(
                                    anew[:, 1024 * p:1024 * (p + 1)],
                                    prs[p][:], AF_LREL,
                                    scale=scv[:, j:j + 1], alpha=0.01)
                            a_prev = anew
                            sig_prev = sig
                            lnv_last = lnv

                        # ---- L5 + un-deferral
                        h5 = p5p.tile([64, 512], f32, name="h5", tag="h5")
                        for tau in range(4):
                            sl = slice(512 * tau, 512 * (tau + 1))
                            nc.tensor.matmul(
                                h5[:],
                                cw["w5b"][:, 64 * tau:64 * (tau + 1)],
                                a_prev[:, sl],
                                start=(tau == 0), stop=False)
                        nc.tensor.matmul(h5[:], cw["b5i"][:], sig_prev[:],
                                         start=False, stop=True)
                        lr5 = opp.tile([64, 512], f32, name="lr5", tag="lr5")
                        nc.scalar.activation(lr5[:], h5[:], AF_LREL, alpha=0.01)
                        rho = opp.tile([64, 512], f32, name="rho", tag="rho")
                        if cfg["rho_exp"]:
                            nc.scalar.activation(rho[:], lnv_last[:], AF.Exp,
                                                 scale=-0.5)
                        else:
                            nc.vector.reciprocal_approx_fast(rho[:], sig_prev[:])
                        oc = opp.tile([64, 512], f32, name="oc", tag="oc")
                        nc.vector.tensor_tensor(oc[:], lr5[:], rho[:], OP.mult)
                        ydst = y_d.rearrange(
                            "(sg sp) (kk tau n) -> sg kk tau sp n",
                            sg=2, kk=4, tau=4, n=512)[g, k]
                        # oc partition-major order (p = 16 tau + sp) matches
                        # the (tau, sp, n) iteration of ydst
                        nc.sync.dma_start(out=ydst, in_=oc[:])

    nc.compile()
    return nc


# ------------------------------------------------------------------- runner

def _get_program():
    key = tuple(sorted(CFG.items()))
    if key not in _CACHE:
        _CACHE[key] = build_program(CFG)
    return _CACHE[key]


def _make_in_maps(inputs):
    x = np.ascontiguousarray(
        np.asarray(inputs["x"], dtype=np.float32).reshape(B_FULL, T))
    md = np.ascontiguousarray(np.asarray(inputs["metadata"], dtype=np.float32))
    wts = _host_weights(inputs)
    in_maps = []
    for i in range(NCORES):
        m = dict(wts)
        m["x"] = np.ascontiguousarray(x[BS * i:BS * (i + 1)])
        m["metadata"] = np.ascontiguousarray(md[BS * i:BS * (i + 1)])
        in_maps.append(m)
    return in_maps


def run_spmd(inputs, trace=False):
    """Run on all 8 cores; returns (y_full, BassKernelResults)."""
    from concourse.bass_utils import run_bass_kernel_spmd
    nc = _get_program()
    in_maps = _make_in_maps(inputs)
    res = run_bass_kernel_spmd(nc, in_maps, core_ids=list(range(NCORES)),
                               trace=trace)
    y = np.concatenate([np.asarray(r["y"]) for r in res.results], axis=0)
    y = y.reshape(B_FULL, 1, T).astype(np.float32)
    return y, res


def kernel(**inputs):
    y, _ = run_spmd(inputs, trace=False)
    return y


# revision 22
# speedup vs baseline: 2.2842x; 1.0082x over previous
"""Trainium2 Bass kernel for nn_AttenuationToRainRate (dense_mlp).

Reference computation per (sample b, position t):
  style MLP: metadata (16) -> 64 -> 128 -> 64, split into 4 x (scale, bias)[8]
  main chain: x -> [w1] -> adain/lrelu -> [w2] -> adain/lrelu -> [w3] ->
              adain/lrelu -> [w4] -> adain/lrelu -> [w5] -> lrelu
  adain(h) = scale * (h - mean_c h) / (std_ddof1(h) + 1e-6) + bias

Design (v3, non-deferred):
  Data-parallel over 8 cores (32 samples each).  Layout: tile [128, 512]
  with partition p = 8*s' + c (16 samples x 8 channels), free = positions.
  Mean-removal is folded into weights host-side (W' = W (I - J/8), b' =
  b - mean b), so d = W' a + b' directly.  Per layer:
    d     : PE matmul (block-diag W') + rank-1 bias (b' row x ones)
    d^2   : ACT Square (PSUM -> SBUF)
    var   : PE matmul with block-ones (channel-sum broadcast), scale 1/7
            folded into the next ACT op
    sigma : ACT Sqrt(var/7 + 1e-12)   [~ matches reference's +1e-6 on std]
    r     : DVE reciprocal_approx_fast(sigma)
    q     : DVE tensor_tensor(d, r)   [d from PSUM]
    a     : ACT Prelu(scale_v * q + bias_v, alpha=0.01)  [per-sample vecs]
  Layer 1 folds its bias via a ones-row appended to the x tile (K=17).
  Layer 5 has no adain: h5 accumulated via banded stationaries into a
  packed [64,512] psum tile (partition 16*tau + s'), out = Prelu(h5+b5).
  All matmul operands are float32r (1 cycle/row at N=512 vs fp32's 4).
  All ACT functions used (Square/Sqrt/Prelu/Relu/Identity) live in the
  sqrt_and_others activation-table set, pinned via a Bacc subclass so
  the table is loaded exactly once.
"""

import numpy as np

B_FULL, T = 256, 8192
NCORES = 8
BS = B_FULL // NCORES  # 32 samples per core
F = 16

# config switches (test.py may flip these and call _reset())
CFG = {
    "mm_r": True,     # float32r matmul operands (1 cyc/row at N>=256)
    "prelu": True,    # Prelu (alpha) for lrelu; False uses Lrelu func
}

_CACHE = {}


def _reset():
    _CACHE.clear()


# ----------------------------------------------------------------- host side

def _host_weights(inp):
    """Weight-derived constants in device layouts (f32 numpy)."""
    f64 = np.float64
    I8 = np.eye(8, dtype=f64)
    C = I8 - np.full((8, 8), 1.0 / 8.0, dtype=f64)  # output-centering

    w = {}
    w1 = np.asarray(inp["w1"], dtype=f64)           # (1, 8)
    b1 = np.asarray(inp["b1"], dtype=f64)           # (8,)
    w1p = (w1 @ C)[0]
    b1p = b1 - b1.mean()
    w1aug = np.zeros((17, 128), dtype=f64)
    for s in range(16):
        w1aug[s, 8 * s:8 * s + 8] = w1p
        w1aug[16, 8 * s:8 * s + 8] = b1p
    w["w1aug"] = w1aug

    brow = np.zeros((1, 3 * 128), dtype=f64)
    for l in (2, 3, 4):
        W = np.asarray(inp[f"w{l}"], dtype=f64) @ C
        bp = np.asarray(inp[f"b{l}"], dtype=f64)
        bp = bp - bp.mean()
        wb = np.zeros((128, 128), dtype=f64)
        for s in range(16):
            wb[8 * s:8 * s + 8, 8 * s:8 * s + 8] = W
        w[f"wb{l}"] = wb
        brow[0, 128 * (l - 2):128 * (l - 1)] = np.tile(bp, 16)
    w["brow"] = brow

    b8 = np.zeros((128, 128), dtype=f64)
    for s in range(16):
        b8[8 * s:8 * s + 8, 8 * s:8 * s + 8] = 1.0
    w["b8bc"] = b8

    w5b = np.zeros((128, 4 * 64), dtype=f64)
    w5 = np.asarray(inp["w5"], dtype=f64)[:, 0]
    for tau in range(4):
        for s in range(16):
            for c in range(8):
                w5b[8 * s + c, 64 * tau + 16 * tau + s] = w5[c]
    w["w5b"] = w5b
    w["b5c"] = np.full((64, 1), float(np.asarray(inp["b5"], dtype=f64)[0]))

    w["onesr"] = np.ones((1, 2048), dtype=f64)
    w["mw1"] = np.asarray(inp["mw1"], dtype=f64)
    w["mw2"] = np.asarray(inp["mw2"], dtype=f64)
    w["mw3"] = np.asarray(inp["mw3"], dtype=f64)
    w["mb1c"] = np.asarray(inp["mb1"], dtype=f64).reshape(64, 1)
    w["mb2c"] = np.asarray(inp["mb2"], dtype=f64).reshape(128, 1)
    w["mb3c"] = np.asarray(inp["mb3"], dtype=f64).reshape(64, 1)

    return {k: np.ascontiguousarray(v, dtype=np.float32) for k, v in w.items()}


_WSHAPES = {
    "w1aug": [17, 128], "brow": [1, 384],
    "wb2": [128, 128], "wb3": [128, 128], "wb4": [128, 128],
    "b8bc": [128, 128], "w5b": [128, 256], "b5c": [64, 1],
    "onesr": [1, 2048],
    "mw1": [16, 64], "mw2": [64, 128], "mw3": [128, 64],
    "mb1c": [64, 1], "mb2c": [128, 1], "mb3c": [64, 1],
}
# tensors that feed PE matmuls (get the float32r dtype)
_MM_STAT = {"w1aug", "brow", "wb2", "wb3", "wb4", "b8bc", "w5b", "onesr"}


# --------------------------------------------------------------- device side

def build_program(cfg=None):
    import concourse.bacc as bacc
    import concourse.mybir as mybir
    from concourse.ap import AP
    from concourse.tile import TileContext

    cfg = dict(CFG if cfg is None else cfg)
    f32 = mybir.dt.float32
    mdt = mybir.dt.float32r if cfg["mm_r"] else f32
    AF = mybir.ActivationFunctionType
    OP = mybir.AluOpType
    AF_LREL = AF.Prelu if cfg["prelu"] else AF.Lrelu

    class _KBacc(bacc.Bacc):
        # The stock insert_act_table_loads greedily picks the FIRST table
        # set containing each activation function, which alternates sets
        # for a Square/Sqrt/Prelu mix -> a ~2.7us ACT_TABLE_LOAD per
        # transition.  Everything we use lives in one set, so blank all
        # other sets (list positions = set ids must be preserved).
        _ACT_SET = "sqrt_and_others"

        def insert_act_table_loads(self):
            import concourse.mybir as _mb
            from concourse.hw_specs import get_activation_tables
            has_activation = any(
                isinstance(i, _mb.InstActivation)
                for b in self.main_func.blocks
                for i in b.instructions
            )
            if not has_activation:
                return
            tables = []
            for name, funcs in get_activation_tables(self.m.arch).items():
                tables.append((name, funcs if name == self._ACT_SET else set()))
            bacc._bass_rust.insert_act_table_loads(self, tables)

    nc = _KBacc("TRN2", target_bir_lowering=False)
    x_d = nc.dram_tensor("x", [BS, T], f32, kind="ExternalInput")
    md_d = nc.dram_tensor("metadata", [BS, F], f32, kind="ExternalInput")
    y_d = nc.dram_tensor("y", [BS, T], f32, kind="ExternalOutput")
    wd = {name: nc.dram_tensor(name, shp, mdt if name in _MM_STAT else f32,
                               kind="ExternalInput")
          for name, shp in _WSHAPES.items()}

    with TileContext(nc) as tc:
        with tc.tile_pool(name="const", bufs=1) as cp, \
             tc.tile_pool(name="scr", bufs=1, space="DRAM") as dp:

            # ---- constants to SBUF
            cw = {}
            for name, shp in _WSHAPES.items():
                t = cp.tile(shp, mdt if name in _MM_STAT else f32,
                            name=f"c_{name}")
                nc.sync.dma_start(out=t[:], in_=wd[name][:])
                cw[name] = t
            eps_s = cp.tile([128, 1], f32, name="eps_s")
            nc.vector.memset(eps_s[:], 1e-12)

            # ---- style MLP (per-core 32 samples)
            with tc.tile_pool(name="stp", bufs=1, space="PSUM") as sp:
                mdT = cp.tile([F, BS], f32, name="mdT")
                nc.sync.dma_start(out=mdT[:], in_=md_d.rearrange("s f -> f s"))
                ps1 = sp.tile([64, BS], f32, name="ps1")
                nc.tensor.matmul(ps1[:], cw["mw1"][:], mdT[:],
                                 start=True, stop=True)
                s1 = cp.tile([64, BS], f32, name="s1")
                nc.scalar.activation(s1[:], ps1[:], AF.Relu, bias=cw["mb1c"][:])
                ps2 = sp.tile([128, BS], f32, name="ps2")
                nc.tensor.matmul(ps2[:], cw["mw2"][:], s1[:],
                                 start=True, stop=True)
                s2 = cp.tile([128, BS], f32, name="s2")
                nc.scalar.activation(s2[:], ps2[:], AF.Relu, bias=cw["mb2c"][:])
                ps3 = sp.tile([64, BS], f32, name="ps3")
                nc.tensor.matmul(ps3[:], cw["mw3"][:], s2[:],
                                 start=True, stop=True)
                sT = cp.tile([64, BS], f32, name="sT")
                nc.scalar.activation(sT[:], ps3[:], AF.Identity,
                                     bias=cw["mb3c"][:])

            # ---- per-(layer, supergroup) scale/bias vectors via DRAM trip
            # sT row = 16(l-1) + 2c + (0 scale / 1 bias), col = 16 sg + s'
            sT_d = dp.tile([64, BS], f32, name="sT_d")
            nc.gpsimd.dma_start(out=sT_d[:], in_=sT[:])
            scv = cp.tile([128, 8], f32, name="scv")   # scale, col j=(l-1)*2+sg
            bcv = cp.tile([128, 8], f32, name="bcv")   # bias
            for l in range(1, 5):
                for g in range(2):
                    j = (l - 1) * 2 + g
                    src_s = AP(tensor=sT_d[:].tensor,
                               offset=512 * (l - 1) + 16 * g,
                               ap=((1, 16), (64, 8)))
                    nc.gpsimd.dma_start(out=scv[:, j:j + 1], in_=src_s)
                    src_b = AP(tensor=sT_d[:].tensor,
                               offset=512 * (l - 1) + 32 + 16 * g,
                               ap=((1, 16), (64, 8)))
                    nc.gpsimd.dma_start(out=bcv[:, j:j + 1], in_=src_b)

            # ---------------- main loop
            with tc.tile_pool(name="pd", bufs=3, space="PSUM") as pdp, \
                 tc.tile_pool(name="pv", bufs=2, space="PSUM") as pvp, \
                 tc.tile_pool(name="xin", bufs=2) as xp, \
                 tc.tile_pool(name="dsqp", bufs=2) as dqp, \
                 tc.tile_pool(name="sgp", bufs=2) as sgp, \
                 tc.tile_pool(name="rpool", bufs=2) as rpp, \
                 tc.tile_pool(name="qpool", bufs=2) as qpp, \
                 tc.tile_pool(name="actp", bufs=2) as app, \
                 tc.tile_pool(name="outp", bufs=2) as opp:

                for g in range(2):
                    for k in range(4):
                        xt = xp.tile([17, 2048], mdt, name="xt", tag="xt")
                        nc.sync.dma_start(
                            out=xt[0:16, :],
                            in_=x_d[16 * g:16 * g + 16,
                                    2048 * k:2048 * (k + 1)].bitcast(mdt))
                        nc.sync.dma_start(out=xt[16:17, :],
                                          in_=cw["onesr"][:])
                        a_prev = None
                        for l in range(1, 5):
                            j = (l - 1) * 2 + g
                            prs = [pdp.tile([128, 1024], f32,
                                            name=f"dt{l}{p}", tag="dt")
                                   for p in range(2)]

                            def dtap(tau):
                                h = tau % 2
                                return prs[tau // 2][:, 512 * h:512 * (h + 1)]

                            for tau in range(4):
                                sl = slice(512 * tau, 512 * (tau + 1))
                                if l == 1:
                                    nc.tensor.matmul(dtap(tau),
                                                     cw["w1aug"][:],
                                                     xt[:, sl],
                                                     start=True, stop=True)
                                else:
                                    nc.tensor.matmul(dtap(tau),
                                                     cw[f"wb{l}"][:],
                                                     a_prev[:, sl],
                                                     start=True, stop=False)
                                    bsl = slice(128 * (l - 2), 128 * (l - 1))
                                    nc.tensor.matmul(dtap(tau),
                                                     cw["brow"][:, bsl],
                                                     cw["onesr"][:, 0:512],
                                                     start=False, stop=True)
                            dsq = dqp.tile([128, 2048], mdt,
                                           name=f"dsq{l}", tag="dsq")
                            for p in range(2):
                                nc.scalar.activation(
                                    dsq[:, 1024 * p:1024 * (p + 1)],
                                    prs[p][:], AF.Square)
                            sg_ = sgp.tile([128, 2048], f32,
                                           name=f"sg{l}", tag="sg")
                            for tau in range(4):
                                sl = slice(512 * tau, 512 * (tau + 1))
                                vb = pvp.tile([128, 512], f32,
                                              name=f"vb{l}{tau}", tag="vb")
                                nc.tensor.matmul(vb[:], cw["b8bc"][:],
                                                 dsq[:, sl],
                                                 start=True, stop=True)
                                # sigma = sqrt(var/7 + eps)
                                nc.scalar.activation(sg_[:, sl], vb[:],
                                                     AF.Sqrt,
                                                     scale=1.0 / 7.0,
                                                     bias=eps_s[:])
                            anew = app.tile([128, 2048], mdt,
                                            name=f"a{l}", tag="a")
                            for p in range(2):
                                psl = slice(1024 * p, 1024 * (p + 1))
                                r_ = rpp.tile([128, 1024], f32,
                                              name=f"r{l}{p}", tag="r")
                                nc.vector.reciprocal_approx_fast(
                                    r_[:], sg_[:, psl])
                                q_ = qpp.tile([128, 1024], f32,
                                              name=f"q{l}{p}", tag="q")
                                nc.vector.tensor_tensor(q_[:], prs[p][:],
                                                        r_[:], OP.mult)
                                nc.scalar.activation(
                                    anew[:, psl], q_[:], AF_LREL,
                                    scale=scv[:, j:j + 1],
                                    bias=bcv[:, j:j + 1], alpha=0.01)
                            a_prev = anew

                        # ---- L5 (no adain): packed [64,512] output
                        h5t = pdp.tile([128, 1024], f32, name="h5", tag="dt")
                        h5 = h5t[0:64, 0:512]
                        for tau in range(4):
                            sl = slice(512 * tau, 512 * (tau + 1))
                            nc.tensor.matmul(
                                h5, cw["w5b"][:, 64 * tau:64 * (tau + 1)],
                                a_prev[:, sl],
                                start=(tau == 0), stop=(tau == 3))
                        oc = opp.tile([64, 512], f32, name="oc", tag="oc")
                        nc.scalar.activation(oc[:], h5, AF_LREL,
                                             bias=cw["b5c"][:], alpha=0.01)
                        ydst = y_d.rearrange(
                            "(sg sp) (kk tau n) -> sg kk tau sp n",
                            sg=2, kk=4, tau=4, n=512)[g, k]
                        # oc partition-major order (p = 16 tau + sp) matches
                        # the (tau, sp, n) iteration of ydst
                        nc.sync.dma_start(out=ydst, in_=oc[:])

    nc.compile()
    return nc


# ------------------------------------------------------------------- runner

def _get_program():
    key = tuple(sorted(CFG.items()))
    if key not in _CACHE:
        _CACHE[key] = build_program(CFG)
    return _CACHE[key]


def _make_in_maps(inputs):
    x = np.ascontiguousarray(
        np.asarray(inputs["x"], dtype=np.float32).reshape(B_FULL, T))
    md = np.ascontiguousarray(np.asarray(inputs["metadata"], dtype=np.float32))
    wts = _host_weights(inputs)
    in_maps = []
    for i in range(NCORES):
        m = dict(wts)
        m["x"] = np.ascontiguousarray(x[BS * i:BS * (i + 1)])
        m["metadata"] = np.ascontiguousarray(md[BS * i:BS * (i + 1)])
        in_maps.append(m)
    return in_maps


def run_spmd(inputs, trace=False):
    """Run on all 8 cores; returns (y_full, BassKernelResults)."""
    from concourse.bass_utils import run_bass_kernel_spmd
    nc = _get_program()
    in_maps = _make_in_maps(inputs)
    res = run_bass_kernel_spmd(nc, in_maps, core_ids=list(range(NCORES)),
                               trace=trace)
    y = np.concatenate([np.asarray(r["y"]) for r in res.results], axis=0)
    y = y.reshape(B_FULL, 1, T).astype(np.float32)
    return y, res


def kernel(**inputs):
    y, _ = run_spmd(inputs, trace=False)
    return y


# revision 23
# speedup vs baseline: 2.5759x; 1.1277x over previous
"""Trainium2 Bass kernel for nn_AttenuationToRainRate (dense_mlp).

Reference computation per (sample b, position t):
  style MLP: metadata (16) -> 64 -> 128 -> 64, split into 4 x (scale, bias)[8]
  main chain: x -> [w1] -> adain/lrelu -> [w2] -> adain/lrelu -> [w3] ->
              adain/lrelu -> [w4] -> adain/lrelu -> [w5] -> lrelu
  adain(h) = scale * (h - mean_c h) / (std_ddof1(h) + 1e-6) + bias

Design (v3, non-deferred):
  Data-parallel over 8 cores (32 samples each).  Layout: tile [128, 512]
  with partition p = 8*s' + c (16 samples x 8 channels), free = positions.
  Mean-removal is folded into weights host-side (W' = W (I - J/8), b' =
  b - mean b), so d = W' a + b' directly.  Per layer:
    d     : PE matmul (block-diag W') + rank-1 bias (b' row x ones)
    d^2   : ACT Square (PSUM -> SBUF)
    var   : PE matmul with block-ones (channel-sum broadcast), scale 1/7
            folded into the next ACT op
    sigma : ACT Sqrt(var/7 + 1e-12)   [~ matches reference's +1e-6 on std]
    r     : DVE reciprocal_approx_fast(sigma)
    q     : DVE tensor_tensor(d, r)   [d from PSUM]
    a     : ACT Prelu(scale_v * q + bias_v, alpha=0.01)  [per-sample vecs]
  Layer 1 folds its bias via a ones-row appended to the x tile (K=17).
  Layer 5 has no adain: h5 accumulated via banded stationaries into a
  packed [64,512] psum tile (partition 16*tau + s'), out = Prelu(h5+b5).
  All matmul operands are float32r (1 cycle/row at N=512 vs fp32's 4).
  All ACT functions used (Square/Sqrt/Prelu/Relu/Identity) live in the
  sqrt_and_others activation-table set, pinned via a Bacc subclass so
  the table is loaded exactly once.
"""

import numpy as np

B_FULL, T = 256, 8192
NCORES = 8
BS = B_FULL // NCORES  # 32 samples per core
F = 16

# config switches (test.py may flip these and call _reset())
CFG = {
    "mm_r": True,     # float32r matmul operands (1 cyc/row at N>=256)
    "prelu": True,    # Prelu (alpha) for lrelu; False uses Lrelu func
}

_CACHE = {}


def _reset():
    _CACHE.clear()


# ----------------------------------------------------------------- host side

def _host_weights(inp):
    """Weight-derived constants in device layouts (f32 numpy)."""
    f64 = np.float64
    I8 = np.eye(8, dtype=f64)
    C = I8 - np.full((8, 8), 1.0 / 8.0, dtype=f64)  # output-centering

    w = {}
    w1 = np.asarray(inp["w1"], dtype=f64)           # (1, 8)
    b1 = np.asarray(inp["b1"], dtype=f64)           # (8,)
    w1p = (w1 @ C)[0]
    b1p = b1 - b1.mean()
    w1aug = np.zeros((17, 128), dtype=f64)
    for s in range(16):
        w1aug[s, 8 * s:8 * s + 8] = w1p
        w1aug[16, 8 * s:8 * s + 8] = b1p
    w["w1aug"] = w1aug

    brow = np.zeros((1, 3 * 128), dtype=f64)
    for l in (2, 3, 4):
        W = np.asarray(inp[f"w{l}"], dtype=f64) @ C
        bp = np.asarray(inp[f"b{l}"], dtype=f64)
        bp = bp - bp.mean()
        wb = np.zeros((128, 128), dtype=f64)
        for s in range(16):
            wb[8 * s:8 * s + 8, 8 * s:8 * s + 8] = W
        w[f"wb{l}"] = wb
        brow[0, 128 * (l - 2):128 * (l - 1)] = np.tile(bp, 16)
    w["brow"] = brow

    b8 = np.zeros((128, 128), dtype=f64)
    for s in range(16):
        b8[8 * s:8 * s + 8, 8 * s:8 * s + 8] = 1.0
    w["b8bc"] = b8

    w5b = np.zeros((128, 4 * 64), dtype=f64)
    w5 = np.asarray(inp["w5"], dtype=f64)[:, 0]
    for tau in range(4):
        for s in range(16):
            for c in range(8):
                w5b[8 * s + c, 64 * tau + 16 * tau + s] = w5[c]
    w["w5b"] = w5b
    w["b5c"] = np.full((64, 1), float(np.asarray(inp["b5"], dtype=f64)[0]))

    w["onesr"] = np.ones((1, 2048), dtype=f64)
    w["mw1"] = np.asarray(inp["mw1"], dtype=f64)
    w["mw2"] = np.asarray(inp["mw2"], dtype=f64)
    w["mw3"] = np.asarray(inp["mw3"], dtype=f64)
    w["mb1c"] = np.asarray(inp["mb1"], dtype=f64).reshape(64, 1)
    w["mb2c"] = np.asarray(inp["mb2"], dtype=f64).reshape(128, 1)
    w["mb3c"] = np.asarray(inp["mb3"], dtype=f64).reshape(64, 1)

    return {k: np.ascontiguousarray(v, dtype=np.float32) for k, v in w.items()}


_WSHAPES = {
    "w1aug": [17, 128], "brow": [1, 384],
    "wb2": [128, 128], "wb3": [128, 128], "wb4": [128, 128],
    "b8bc": [128, 128], "w5b": [128, 256], "b5c": [64, 1],
    "onesr": [1, 2048],
    "mw1": [16, 64], "mw2": [64, 128], "mw3": [128, 64],
    "mb1c": [64, 1], "mb2c": [128, 1], "mb3c": [64, 1],
}
# tensors that feed PE matmuls (get the float32r dtype)
_MM_STAT = {"w1aug", "brow", "wb2", "wb3", "wb4", "b8bc", "w5b", "onesr"}


# --------------------------------------------------------------- device side

def build_program(cfg=None):
    import concourse.bacc as bacc
    import concourse.mybir as mybir
    from concourse.ap import AP
    from concourse.tile import TileContext

    cfg = dict(CFG if cfg is None else cfg)
    f32 = mybir.dt.float32
    mdt = mybir.dt.float32r if cfg["mm_r"] else f32
    AF = mybir.ActivationFunctionType
    OP = mybir.AluOpType
    AF_LREL = AF.Prelu if cfg["prelu"] else AF.Lrelu

    class _KBacc(bacc.Bacc):
        # The stock insert_act_table_loads greedily picks the FIRST table
        # set containing each activation function, which alternates sets
        # for a Square/Sqrt/Prelu mix -> a ~2.7us ACT_TABLE_LOAD per
        # transition.  Everything we use lives in one set, so blank all
        # other sets (list positions = set ids must be preserved).
        _ACT_SET = "abs_reciprocal_sqrt_and_small"

        def insert_act_table_loads(self):
            import concourse.mybir as _mb
            from concourse.hw_specs import get_activation_tables
            has_activation = any(
                isinstance(i, _mb.InstActivation)
                for b in self.main_func.blocks
                for i in b.instructions
            )
            if not has_activation:
                return
            tables = []
            for name, funcs in get_activation_tables(self.m.arch).items():
                tables.append((name, funcs if name == self._ACT_SET else set()))
            bacc._bass_rust.insert_act_table_loads(self, tables)

    nc = _KBacc("TRN2", target_bir_lowering=False)
    x_d = nc.dram_tensor("x", [BS, T], f32, kind="ExternalInput")
    md_d = nc.dram_tensor("metadata", [BS, F], f32, kind="ExternalInput")
    y_d = nc.dram_tensor("y", [BS, T], f32, kind="ExternalOutput")
    wd = {name: nc.dram_tensor(name, shp, mdt if name in _MM_STAT else f32,
                               kind="ExternalInput")
          for name, shp in _WSHAPES.items()}

    with TileContext(nc) as tc:
        with tc.tile_pool(name="const", bufs=1) as cp, \
             tc.tile_pool(name="scr", bufs=1, space="DRAM") as dp:

            # ---- constants to SBUF
            cw = {}
            for name, shp in _WSHAPES.items():
                t = cp.tile(shp, mdt if name in _MM_STAT else f32,
                            name=f"c_{name}")
                nc.sync.dma_start(out=t[:], in_=wd[name][:])
                cw[name] = t
            eps_s = cp.tile([128, 1], f32, name="eps_s")
            nc.vector.memset(eps_s[:], 1e-12)

            # ---- style MLP (per-core 32 samples)
            with tc.tile_pool(name="stp", bufs=1, space="PSUM") as sp:
                mdT = cp.tile([F, BS], f32, name="mdT")
                nc.sync.dma_start(out=mdT[:], in_=md_d.rearrange("s f -> f s"))
                ps1 = sp.tile([64, BS], f32, name="ps1")
                nc.tensor.matmul(ps1[:], cw["mw1"][:], mdT[:],
                                 start=True, stop=True)
                s1 = cp.tile([64, BS], f32, name="s1")
                nc.scalar.activation(s1[:], ps1[:], AF.Relu, bias=cw["mb1c"][:])
                ps2 = sp.tile([128, BS], f32, name="ps2")
                nc.tensor.matmul(ps2[:], cw["mw2"][:], s1[:],
                                 start=True, stop=True)
                s2 = cp.tile([128, BS], f32, name="s2")
                nc.scalar.activation(s2[:], ps2[:], AF.Relu, bias=cw["mb2c"][:])
                ps3 = sp.tile([64, BS], f32, name="ps3")
                nc.tensor.matmul(ps3[:], cw["mw3"][:], s2[:],
                                 start=True, stop=True)
                sT = cp.tile([64, BS], f32, name="sT")
                nc.scalar.activation(sT[:], ps3[:], AF.Identity,
                                     bias=cw["mb3c"][:])

            # ---- per-(layer, supergroup) scale/bias vectors via DRAM trip
            # sT row = 16(l-1) + 2c + (0 scale / 1 bias), col = 16 sg + s'
            sT_d = dp.tile([64, BS], f32, name="sT_d")
            nc.gpsimd.dma_start(out=sT_d[:], in_=sT[:])
            scv = cp.tile([128, 8], f32, name="scv")   # scale, col j=(l-1)*2+sg
            bcv = cp.tile([128, 8], f32, name="bcv")   # bias
            for l in range(1, 5):
                for g in range(2):
                    j = (l - 1) * 2 + g
                    src_s = AP(tensor=sT_d[:].tensor,
                               offset=512 * (l - 1) + 16 * g,
                               ap=((1, 16), (64, 8)))
                    nc.gpsimd.dma_start(out=scv[:, j:j + 1], in_=src_s)
                    src_b = AP(tensor=sT_d[:].tensor,
                               offset=512 * (l - 1) + 32 + 16 * g,
                               ap=((1, 16), (64, 8)))
                    nc.gpsimd.dma_start(out=bcv[:, j:j + 1], in_=src_b)

            # ---------------- main loop
            with tc.tile_pool(name="pd", bufs=3, space="PSUM") as pdp, \
                 tc.tile_pool(name="pv", bufs=1, space="PSUM") as pvp, \
                 tc.tile_pool(name="xin", bufs=2) as xp, \
                 tc.tile_pool(name="dsqp", bufs=2) as dqp, \
                 tc.tile_pool(name="sgp", bufs=2) as sgp, \
                 tc.tile_pool(name="rpool", bufs=2) as rpp, \
                 tc.tile_pool(name="qpool", bufs=2) as qpp, \
                 tc.tile_pool(name="actp", bufs=2) as app, \
                 tc.tile_pool(name="outp", bufs=2) as opp:

                for g in range(2):
                    for k in range(4):
                        xt = xp.tile([17, 2048], mdt, name="xt", tag="xt")
                        nc.sync.dma_start(
                            out=xt[0:16, :],
                            in_=x_d[16 * g:16 * g + 16,
                                    2048 * k:2048 * (k + 1)].bitcast(mdt))
                        nc.sync.dma_start(out=xt[16:17, :],
                                          in_=cw["onesr"][:])
                        a_prev = None
                        for l in range(1, 5):
                            j = (l - 1) * 2 + g
                            prs = [pdp.tile([128, 1024], f32,
                                            name=f"dt{l}{p}", tag="dt")
                                   for p in range(2)]

                            def dtap(tau):
                                h = tau % 2
                                return prs[tau // 2][:, 512 * h:512 * (h + 1)]

                            for tau in range(4):
                                sl = slice(512 * tau, 512 * (tau + 1))
                                if l == 1:
                                    nc.tensor.matmul(dtap(tau),
                                                     cw["w1aug"][:],
                                                     xt[:, sl],
                                                     start=True, stop=True)
                                else:
                                    nc.tensor.matmul(dtap(tau),
                                                     cw[f"wb{l}"][:],
                                                     a_prev[:, sl],
                                                     start=True, stop=False)
                                    bsl = slice(128 * (l - 2), 128 * (l - 1))
                                    nc.tensor.matmul(dtap(tau),
                                                     cw["brow"][:, bsl],
                                                     cw["onesr"][:, 0:512],
                                                     start=False, stop=True)
                            dsq = dqp.tile([128, 2048], mdt,
                                           name=f"dsq{l}", tag="dsq")
                            for p in range(2):
                                nc.scalar.activation(
                                    dsq[:, 1024 * p:1024 * (p + 1)],
                                    prs[p][:], AF.Square)
                            anew = app.tile([128, 2048], mdt,
                                            name=f"a{l}", tag="a")
                            for p in range(2):
                                psl = slice(1024 * p, 1024 * (p + 1))
                                vb = pvp.tile([128, 1024], f32,
                                              name=f"vb{l}{p}", tag="vb")
                                for h in range(2):
                                    tau = 2 * p + h
                                    nc.tensor.matmul(
                                        vb[:, 512 * h:512 * (h + 1)],
                                        cw["b8bc"][:],
                                        dsq[:, 512 * tau:512 * (tau + 1)],
                                        start=True, stop=True)
                                # r = 1/sigma = (|var/7 + eps|)^-1/2
                                r_ = rpp.tile([128, 1024], f32,
                                              name=f"r{l}{p}", tag="r")
                                nc.scalar.activation(r_[:], vb[:],
                                                     AF.Abs_reciprocal_sqrt,
                                                     scale=1.0 / 7.0,
                                                     bias=eps_s[:])
                                q_ = qpp.tile([128, 1024], f32,
                                              name=f"q{l}{p}", tag="q")
                                nc.vector.tensor_tensor(q_[:], prs[p][:],
                                                        r_[:], OP.mult)
                                nc.scalar.activation(
                                    anew[:, psl], q_[:], AF_LREL,
                                    scale=scv[:, j:j + 1],
                                    bias=bcv[:, j:j + 1], alpha=0.01)
                            a_prev = anew

                        # ---- L5 (no adain): packed [64,512] output
                        h5t = pdp.tile([128, 1024], f32, name="h5", tag="dt")
                        h5 = h5t[0:64, 0:512]
                        for tau in range(4):
                            sl = slice(512 * tau, 512 * (tau + 1))
                            nc.tensor.matmul(
                                h5, cw["w5b"][:, 64 * tau:64 * (tau + 1)],
                                a_prev[:, sl],
                                start=(tau == 0), stop=(tau == 3))
                        oc = opp.tile([64, 512], f32, name="oc", tag="oc")
                        nc.scalar.activation(oc[:], h5, AF_LREL,
                                             bias=cw["b5c"][:], alpha=0.01)
                        ydst = y_d.rearrange(
                            "(sg sp) (kk tau n) -> sg kk tau sp n",
                            sg=2, kk=4, tau=4, n=512)[g, k]
                        # oc partition-major order (p = 16 tau + sp) matches
                        # the (tau, sp, n) iteration of ydst
                        nc.sync.dma_start(out=ydst, in_=oc[:])

    nc.compile()
    return nc


# ------------------------------------------------------------------- runner

def _get_program():
    key = tuple(sorted(CFG.items()))
    if key not in _CACHE:
        _CACHE[key] = build_program(CFG)
    return _CACHE[key]


def _make_in_maps(inputs):
    x = np.ascontiguousarray(
        np.asarray(inputs["x"], dtype=np.float32).reshape(B_FULL, T))
    md = np.ascontiguousarray(np.asarray(inputs["metadata"], dtype=np.float32))
    wts = _host_weights(inputs)
    in_maps = []
    for i in range(NCORES):
        m = dict(wts)
        m["x"] = np.ascontiguousarray(x[BS * i:BS * (i + 1)])
        m["metadata"] = np.ascontiguousarray(md[BS * i:BS * (i + 1)])
        in_maps.append(m)
    return in_maps


def run_spmd(inputs, trace=False):
    """Run on all 8 cores; returns (y_full, BassKernelResults)."""
    from concourse.bass_utils import run_bass_kernel_spmd
    nc = _get_program()
    in_maps = _make_in_maps(inputs)
    res = run_bass_kernel_spmd(nc, in_maps, core_ids=list(range(NCORES)),
                               trace=trace)
    y = np.concatenate([np.asarray(r["y"]) for r in res.results], axis=0)
    y = y.reshape(B_FULL, 1, T).astype(np.float32)
    return y, res


def kernel(**inputs):
    y, _ = run_spmd(inputs, trace=False)
    return y


# revision 24
# speedup vs baseline: 2.9512x; 1.1457x over previous
"""Trainium2 Bass kernel for nn_AttenuationToRainRate (dense_mlp).

Reference computation per (sample b, position t):
  style MLP: metadata (16) -> 64 -> 128 -> 64, split into 4 x (scale, bias)[8]
  main chain: x -> [w1] -> adain/lrelu -> [w2] -> adain/lrelu -> [w3] ->
              adain/lrelu -> [w4] -> adain/lrelu -> [w5] -> lrelu
  adain(h) = scale * (h - mean_c h) / (std_ddof1(h) + 1e-6) + bias

Design (v3, non-deferred):
  Data-parallel over 8 cores (32 samples each).  Layout: tile [128, 512]
  with partition p = 8*s' + c (16 samples x 8 channels), free = positions.
  Mean-removal is folded into weights host-side (W' = W (I - J/8), b' =
  b - mean b), so d = W' a + b' directly.  Per layer:
    d     : PE matmul (block-diag W') + rank-1 bias (b' row x ones)
    d^2   : ACT Square (PSUM -> SBUF)
    var   : PE matmul with block-ones (channel-sum broadcast), scale 1/7
            folded into the next ACT op
    sigma : ACT Sqrt(var/7 + 1e-12)   [~ matches reference's +1e-6 on std]
    r     : DVE reciprocal_approx_fast(sigma)
    q     : DVE tensor_tensor(d, r)   [d from PSUM]
    a     : ACT Prelu(scale_v * q + bias_v, alpha=0.01)  [per-sample vecs]
  Layer 1 folds its bias via a ones-row appended to the x tile (K=17).
  Layer 5 has no adain: h5 accumulated via banded stationaries into a
  packed [64,512] psum tile (partition 16*tau + s'), out = Prelu(h5+b5).
  All matmul operands are float32r (1 cycle/row at N=512 vs fp32's 4).
  All ACT functions used (Square/Sqrt/Prelu/Relu/Identity) live in the
  sqrt_and_others activation-table set, pinned via a Bacc subclass so
  the table is loaded exactly once.
"""

import numpy as np

B_FULL, T = 256, 8192
NCORES = 8
BS = B_FULL // NCORES  # 32 samples per core
F = 16

# config switches (test.py may flip these and call _reset())
CFG = {
    "mm_dt": "fp16",  # matmul operand dtype: fp16 (FWL fast weight load,
                      # 1 cyc/row) | f32r (1 cyc/row, slow 4B LDWEIGHTS) | f32
    "prelu": True,    # Prelu (alpha) for lrelu; False uses Lrelu func
}

_CACHE = {}


def _reset():
    _CACHE.clear()


# ----------------------------------------------------------------- host side

def _host_weights(inp):
    """Weight-derived constants in device layouts (f32 numpy)."""
    f64 = np.float64
    I8 = np.eye(8, dtype=f64)
    C = I8 - np.full((8, 8), 1.0 / 8.0, dtype=f64)  # output-centering

    w = {}
    w1 = np.asarray(inp["w1"], dtype=f64)           # (1, 8)
    b1 = np.asarray(inp["b1"], dtype=f64)           # (8,)
    w1p = (w1 @ C)[0]
    b1p = b1 - b1.mean()
    w1aug = np.zeros((17, 128), dtype=f64)
    for s in range(16):
        w1aug[s, 8 * s:8 * s + 8] = w1p
        w1aug[16, 8 * s:8 * s + 8] = b1p
    w["w1aug"] = w1aug

    brow = np.zeros((1, 3 * 128), dtype=f64)
    for l in (2, 3, 4):
        W = np.asarray(inp[f"w{l}"], dtype=f64) @ C
        bp = np.asarray(inp[f"b{l}"], dtype=f64)
        bp = bp - bp.mean()
        wb = np.zeros((128, 128), dtype=f64)
        for s in range(16):
            wb[8 * s:8 * s + 8, 8 * s:8 * s + 8] = W
        w[f"wb{l}"] = wb
        brow[0, 128 * (l - 2):128 * (l - 1)] = np.tile(bp, 16)
    w["brow"] = brow

    b8 = np.zeros((128, 128), dtype=f64)
    for s in range(16):
        b8[8 * s:8 * s + 8, 8 * s:8 * s + 8] = 1.0
    w["b8bc"] = b8

    w5b = np.zeros((128, 4 * 64), dtype=f64)
    w5 = np.asarray(inp["w5"], dtype=f64)[:, 0]
    for tau in range(4):
        for s in range(16):
            for c in range(8):
                w5b[8 * s + c, 64 * tau + 16 * tau + s] = w5[c]
    w["w5b"] = w5b
    w["b5c"] = np.full((64, 1), float(np.asarray(inp["b5"], dtype=f64)[0]))

    w["onesr"] = np.ones((1, 2048), dtype=f64)
    w["mw1"] = np.asarray(inp["mw1"], dtype=f64)
    w["mw2"] = np.asarray(inp["mw2"], dtype=f64)
    w["mw3"] = np.asarray(inp["mw3"], dtype=f64)
    w["mb1c"] = np.asarray(inp["mb1"], dtype=f64).reshape(64, 1)
    w["mb2c"] = np.asarray(inp["mb2"], dtype=f64).reshape(128, 1)
    w["mb3c"] = np.asarray(inp["mb3"], dtype=f64).reshape(64, 1)

    mm_np = {"fp16": np.float16, "f32r": np.float32, "f32": np.float32}[
        CFG["mm_dt"]]
    out = {}
    for k, v in w.items():
        dt = mm_np if k in _MM_STAT else np.float32
        out[k] = np.ascontiguousarray(v.astype(dt))
    return out


_WSHAPES = {
    "w1aug": [17, 128], "brow": [1, 384],
    "wb2": [128, 128], "wb3": [128, 128], "wb4": [128, 128],
    "b8bc": [128, 128], "w5b": [128, 256], "b5c": [64, 1],
    "onesr": [1, 2048],
    "mw1": [16, 64], "mw2": [64, 128], "mw3": [128, 64],
    "mb1c": [64, 1], "mb2c": [128, 1], "mb3c": [64, 1],
}
# tensors that feed PE matmuls (get the float32r dtype)
_MM_STAT = {"w1aug", "brow", "wb2", "wb3", "wb4", "b8bc", "w5b", "onesr"}


# --------------------------------------------------------------- device side

def build_program(cfg=None):
    import concourse.bacc as bacc
    import concourse.mybir as mybir
    from concourse.ap import AP
    from concourse.tile import TileContext

    cfg = dict(CFG if cfg is None else cfg)
    f32 = mybir.dt.float32
    mdt = {"fp16": mybir.dt.float16, "f32r": mybir.dt.float32r,
           "f32": f32}[cfg["mm_dt"]]
    AF = mybir.ActivationFunctionType
    OP = mybir.AluOpType
    AF_LREL = AF.Prelu if cfg["prelu"] else AF.Lrelu

    class _KBacc(bacc.Bacc):
        # The stock insert_act_table_loads greedily picks the FIRST table
        # set containing each activation function, which alternates sets
        # for a Square/Sqrt/Prelu mix -> a ~2.7us ACT_TABLE_LOAD per
        # transition.  Everything we use lives in one set, so blank all
        # other sets (list positions = set ids must be preserved).
        _ACT_SET = "abs_reciprocal_sqrt_and_small"

        def insert_act_table_loads(self):
            import concourse.mybir as _mb
            from concourse.hw_specs import get_activation_tables
            has_activation = any(
                isinstance(i, _mb.InstActivation)
                for b in self.main_func.blocks
                for i in b.instructions
            )
            if not has_activation:
                return
            tables = []
            for name, funcs in get_activation_tables(self.m.arch).items():
                tables.append((name, funcs if name == self._ACT_SET else set()))
            bacc._bass_rust.insert_act_table_loads(self, tables)

    nc = _KBacc("TRN2", target_bir_lowering=False)
    x_d = nc.dram_tensor("x", [BS, T], mdt, kind="ExternalInput")
    md_d = nc.dram_tensor("metadata", [BS, F], f32, kind="ExternalInput")
    y_d = nc.dram_tensor("y", [BS, T], f32, kind="ExternalOutput")
    wd = {name: nc.dram_tensor(name, shp, mdt if name in _MM_STAT else f32,
                               kind="ExternalInput")
          for name, shp in _WSHAPES.items()}

    with TileContext(nc) as tc:
        with tc.tile_pool(name="const", bufs=1) as cp, \
             tc.tile_pool(name="scr", bufs=1, space="DRAM") as dp:

            # ---- constants to SBUF
            cw = {}
            for name, shp in _WSHAPES.items():
                t = cp.tile(shp, mdt if name in _MM_STAT else f32,
                            name=f"c_{name}")
                nc.sync.dma_start(out=t[:], in_=wd[name][:])
                cw[name] = t
            eps_s = cp.tile([128, 1], f32, name="eps_s")
            nc.vector.memset(eps_s[:], 1e-12)

            # ---- style MLP (per-core 32 samples)
            with tc.tile_pool(name="stp", bufs=1, space="PSUM") as sp:
                mdT = cp.tile([F, BS], f32, name="mdT")
                nc.sync.dma_start(out=mdT[:], in_=md_d.rearrange("s f -> f s"))
                ps1 = sp.tile([64, BS], f32, name="ps1")
                nc.tensor.matmul(ps1[:], cw["mw1"][:], mdT[:],
                                 start=True, stop=True)
                s1 = cp.tile([64, BS], f32, name="s1")
                nc.scalar.activation(s1[:], ps1[:], AF.Relu, bias=cw["mb1c"][:])
                ps2 = sp.tile([128, BS], f32, name="ps2")
                nc.tensor.matmul(ps2[:], cw["mw2"][:], s1[:],
                                 start=True, stop=True)
                s2 = cp.tile([128, BS], f32, name="s2")
                nc.scalar.activation(s2[:], ps2[:], AF.Relu, bias=cw["mb2c"][:])
                ps3 = sp.tile([64, BS], f32, name="ps3")
                nc.tensor.matmul(ps3[:], cw["mw3"][:], s2[:],
                                 start=True, stop=True)
                sT = cp.tile([64, BS], f32, name="sT")
                nc.scalar.activation(sT[:], ps3[:], AF.Identity,
                                     bias=cw["mb3c"][:])

            # ---- per-(layer, supergroup) scale/bias vectors via DRAM trip
            # sT row = 16(l-1) + 2c + (0 scale / 1 bias), col = 16 sg + s'
            sT_d = dp.tile([64, BS], f32, name="sT_d")
            nc.gpsimd.dma_start(out=sT_d[:], in_=sT[:])
            scv = cp.tile([128, 8], f32, name="scv")   # scale, col j=(l-1)*2+sg
            bcv = cp.tile([128, 8], f32, name="bcv")   # bias
            for l in range(1, 5):
                for g in range(2):
                    j = (l - 1) * 2 + g
                    src_s = AP(tensor=sT_d[:].tensor,
                               offset=512 * (l - 1) + 16 * g,
                               ap=((1, 16), (64, 8)))
                    nc.gpsimd.dma_start(out=scv[:, j:j + 1], in_=src_s)
                    src_b = AP(tensor=sT_d[:].tensor,
                               offset=512 * (l - 1) + 32 + 16 * g,
                               ap=((1, 16), (64, 8)))
                    nc.gpsimd.dma_start(out=bcv[:, j:j + 1], in_=src_b)

            # ---------------- main loop
            with tc.tile_pool(name="pd", bufs=3, space="PSUM") as pdp, \
                 tc.tile_pool(name="pv", bufs=1, space="PSUM") as pvp, \
                 tc.tile_pool(name="xin", bufs=2) as xp, \
                 tc.tile_pool(name="dsqp", bufs=2) as dqp, \
                 tc.tile_pool(name="sgp", bufs=2) as sgp, \
                 tc.tile_pool(name="rpool", bufs=2) as rpp, \
                 tc.tile_pool(name="qpool", bufs=2) as qpp, \
                 tc.tile_pool(name="actp", bufs=2) as app, \
                 tc.tile_pool(name="outp", bufs=2) as opp:

                for g in range(2):
                    for k in range(4):
                        xt = xp.tile([17, 2048], mdt, name="xt", tag="xt")
                        nc.sync.dma_start(
                            out=xt[0:16, :],
                            in_=x_d[16 * g:16 * g + 16,
                                    2048 * k:2048 * (k + 1)])
                        nc.sync.dma_start(out=xt[16:17, :],
                                          in_=cw["onesr"][:])
                        a_prev = None
                        for l in range(1, 5):
                            j = (l - 1) * 2 + g
                            prs = [pdp.tile([128, 1024], f32,
                                            name=f"dt{l}{p}", tag="dt")
                                   for p in range(2)]

                            def dtap(tau):
                                h = tau % 2
                                return prs[tau // 2][:, 512 * h:512 * (h + 1)]

                            for tau in range(4):
                                sl = slice(512 * tau, 512 * (tau + 1))
                                if l == 1:
                                    nc.tensor.matmul(dtap(tau),
                                                     cw["w1aug"][:],
                                                     xt[:, sl],
                                                     start=True, stop=True)
                                else:
                                    nc.tensor.matmul(dtap(tau),
                                                     cw[f"wb{l}"][:],
                                                     a_prev[:, sl],
                                                     start=True, stop=False)
                                    bsl = slice(128 * (l - 2), 128 * (l - 1))
                                    nc.tensor.matmul(dtap(tau),
                                                     cw["brow"][:, bsl],
                                                     cw["onesr"][:, 0:512],
                                                     start=False, stop=True)
                            dsq = dqp.tile([128, 2048], mdt,
                                           name=f"dsq{l}", tag="dsq")
                            for p in range(2):
                                nc.scalar.activation(
                                    dsq[:, 1024 * p:1024 * (p + 1)],
                                    prs[p][:], AF.Square)
                            anew = app.tile([128, 2048], mdt,
                                            name=f"a{l}", tag="a")
                            for p in range(2):
                                psl = slice(1024 * p, 1024 * (p + 1))
                                vb = pvp.tile([128, 1024], f32,
                                              name=f"vb{l}{p}", tag="vb")
                                for h in range(2):
                                    tau = 2 * p + h
                                    nc.tensor.matmul(
                                        vb[:, 512 * h:512 * (h + 1)],
                                        cw["b8bc"][:],
                                        dsq[:, 512 * tau:512 * (tau + 1)],
                                        start=True, stop=True)
                                # r = 1/sigma = (|var/7 + eps|)^-1/2
                                r_ = rpp.tile([128, 1024], f32,
                                              name=f"r{l}{p}", tag="r")
                                nc.scalar.activation(r_[:], vb[:],
                                                     AF.Abs_reciprocal_sqrt,
                                                     scale=1.0 / 7.0,
                                                     bias=eps_s[:])
                                q_ = qpp.tile([128, 1024], f32,
                                              name=f"q{l}{p}", tag="q")
                                nc.vector.tensor_tensor(q_[:], prs[p][:],
                                                        r_[:], OP.mult)
                                nc.scalar.activation(
                                    anew[:, psl], q_[:], AF_LREL,
                                    scale=scv[:, j:j + 1],
                                    bias=bcv[:, j:j + 1], alpha=0.01)
                            a_prev = anew

                        # ---- L5 (no adain): packed [64,512] output
                        h5t = pdp.tile([128, 1024], f32, name="h5", tag="dt")
                        h5 = h5t[0:64, 0:512]
                        for tau in range(4):
                            sl = slice(512 * tau, 512 * (tau + 1))
                            nc.tensor.matmul(
                                h5, cw["w5b"][:, 64 * tau:64 * (tau + 1)],
                                a_prev[:, sl],
                                start=(tau == 0), stop=(tau == 3))
                        oc = opp.tile([64, 512], f32, name="oc", tag="oc")
                        nc.scalar.activation(oc[:], h5, AF_LREL,
                                             bias=cw["b5c"][:], alpha=0.01)
                        ydst = y_d.rearrange(
                            "(sg sp) (kk tau n) -> sg kk tau sp n",
                            sg=2, kk=4, tau=4, n=512)[g, k]
                        # oc partition-major order (p = 16 tau + sp) matches
                        # the (tau, sp, n) iteration of ydst
                        nc.sync.dma_start(out=ydst, in_=oc[:])

    nc.compile()
    return nc


# ------------------------------------------------------------------- runner

def _get_program():
    key = tuple(sorted(CFG.items()))
    if key not in _CACHE:
        _CACHE[key] = build_program(CFG)
    return _CACHE[key]


def _make_in_maps(inputs):
    mm_np = {"fp16": np.float16, "f32r": np.float32, "f32": np.float32}[
        CFG["mm_dt"]]
    x = np.ascontiguousarray(
        np.asarray(inputs["x"], dtype=np.float32).reshape(B_FULL, T).astype(
            mm_np))
    md = np.ascontiguousarray(np.asarray(inputs["metadata"], dtype=np.float32))
    wts = _host_weights(inputs)
    in_maps = []
    for i in range(NCORES):
        m = dict(wts)
        m["x"] = np.ascontiguousarray(x[BS * i:BS * (i + 1)])
        m["metadata"] = np.ascontiguousarray(md[BS * i:BS * (i + 1)])
        in_maps.append(m)
    return in_maps


def run_spmd(inputs, trace=False):
    """Run on all 8 cores; returns (y_full, BassKernelResults)."""
    from concourse.bass_utils import run_bass_kernel_spmd
    nc = _get_program()
    in_maps = _make_in_maps(inputs)
    res = run_bass_kernel_spmd(nc, in_maps, core_ids=list(range(NCORES)),
                               trace=trace)
    y = np.concatenate([np.asarray(r["y"]) for r in res.results], axis=0)
    y = y.reshape(B_FULL, 1, T).astype(np.float32)
    return y, res


def kernel(**inputs):
    y, _ = run_spmd(inputs, trace=False)
    return y


# revision 25
# speedup vs baseline: 2.9521x; 1.0003x over previous
"""Trainium2 Bass kernel for nn_AttenuationToRainRate (dense_mlp).

Reference computation per (sample b, position t):
  style MLP: metadata (16) -> 64 -> 128 -> 64, split into 4 x (scale, bias)[8]
  main chain: x -> [w1] -> adain/lrelu -> [w2] -> adain/lrelu -> [w3] ->
              adain/lrelu -> [w4] -> adain/lrelu -> [w5] -> lrelu
  adain(h) = scale * (h - mean_c h) / (std_ddof1(h) + 1e-6) + bias

Design (v3, non-deferred):
  Data-parallel over 8 cores (32 samples each).  Layout: tile [128, 512]
  with partition p = 8*s' + c (16 samples x 8 channels), free = positions.
  Mean-removal is folded into weights host-side (W' = W (I - J/8), b' =
  b - mean b), so d = W' a + b' directly.  Per layer:
    d     : PE matmul (block-diag W') + rank-1 bias (b' row x ones)
    d^2   : ACT Square (PSUM -> SBUF)
    var   : PE matmul with block-ones (channel-sum broadcast), scale 1/7
            folded into the next ACT op
    sigma : ACT Sqrt(var/7 + 1e-12)   [~ matches reference's +1e-6 on std]
    r     : DVE reciprocal_approx_fast(sigma)
    q     : DVE tensor_tensor(d, r)   [d from PSUM]
    a     : ACT Prelu(scale_v * q + bias_v, alpha=0.01)  [per-sample vecs]
  Layer 1 folds its bias via a ones-row appended to the x tile (K=17).
  Layer 5 has no adain: h5 accumulated via banded stationaries into a
  packed [64,512] psum tile (partition 16*tau + s'), out = Prelu(h5+b5).
  All matmul operands are float32r (1 cycle/row at N=512 vs fp32's 4).
  All ACT functions used (Square/Sqrt/Prelu/Relu/Identity) live in the
  sqrt_and_others activation-table set, pinned via a Bacc subclass so
  the table is loaded exactly once.
"""

import numpy as np

B_FULL, T = 256, 8192
NCORES = 8
BS = B_FULL // NCORES  # 32 samples per core
F = 16

# config switches (test.py may flip these and call _reset())
CFG = {
    "mm_dt": "fp16",  # matmul operand dtype: fp16 (FWL fast weight load,
                      # 1 cyc/row) | f32r (1 cyc/row, slow 4B LDWEIGHTS) | f32
    "prelu": True,    # Prelu (alpha) for lrelu; False uses Lrelu func
}

_CACHE = {}


def _reset():
    _CACHE.clear()


# ----------------------------------------------------------------- host side

def _host_weights(inp):
    """Weight-derived constants in device layouts (f32 numpy)."""
    f64 = np.float64
    I8 = np.eye(8, dtype=f64)
    C = I8 - np.full((8, 8), 1.0 / 8.0, dtype=f64)  # output-centering

    w = {}
    w1 = np.asarray(inp["w1"], dtype=f64)           # (1, 8)
    b1 = np.asarray(inp["b1"], dtype=f64)           # (8,)
    w1p = (w1 @ C)[0]
    b1p = b1 - b1.mean()
    w1aug = np.zeros((17, 128), dtype=f64)
    for s in range(16):
        w1aug[s, 8 * s:8 * s + 8] = w1p
        w1aug[16, 8 * s:8 * s + 8] = b1p
    w["w1aug"] = w1aug

    brow = np.zeros((1, 3 * 128), dtype=f64)
    for l in (2, 3, 4):
        W = np.asarray(inp[f"w{l}"], dtype=f64) @ C
        bp = np.asarray(inp[f"b{l}"], dtype=f64)
        bp = bp - bp.mean()
        wb = np.zeros((128, 128), dtype=f64)
        for s in range(16):
            wb[8 * s:8 * s + 8, 8 * s:8 * s + 8] = W
        w[f"wb{l}"] = wb
        brow[0, 128 * (l - 2):128 * (l - 1)] = np.tile(bp, 16)
    w["brow"] = brow

    b8 = np.zeros((128, 128), dtype=f64)
    for s in range(16):
        b8[8 * s:8 * s + 8, 8 * s:8 * s + 8] = 1.0
    w["b8bc"] = b8

    w5b = np.zeros((128, 4 * 64), dtype=f64)
    w5 = np.asarray(inp["w5"], dtype=f64)[:, 0]
    for tau in range(4):
        for s in range(16):
            for c in range(8):
                w5b[8 * s + c, 64 * tau + 16 * tau + s] = w5[c]
    w["w5b"] = w5b
    w["b5c"] = np.full((64, 1), float(np.asarray(inp["b5"], dtype=f64)[0]))

    w["onesr"] = np.ones((1, 2048), dtype=f64)
    w["mw1"] = np.asarray(inp["mw1"], dtype=f64)
    w["mw2"] = np.asarray(inp["mw2"], dtype=f64)
    w["mw3"] = np.asarray(inp["mw3"], dtype=f64)
    w["mb1c"] = np.asarray(inp["mb1"], dtype=f64).reshape(64, 1)
    w["mb2c"] = np.asarray(inp["mb2"], dtype=f64).reshape(128, 1)
    w["mb3c"] = np.asarray(inp["mb3"], dtype=f64).reshape(64, 1)

    mm_np = {"fp16": np.float16, "f32r": np.float32, "f32": np.float32}[
        CFG["mm_dt"]]
    out = {}
    for k, v in w.items():
        dt = mm_np if k in _MM_STAT else np.float32
        out[k] = np.ascontiguousarray(v.astype(dt))
    return out


_WSHAPES = {
    "w1aug": [17, 128], "brow": [1, 384],
    "wb2": [128, 128], "wb3": [128, 128], "wb4": [128, 128],
    "b8bc": [128, 128], "w5b": [128, 256], "b5c": [64, 1],
    "onesr": [1, 2048],
    "mw1": [16, 64], "mw2": [64, 128], "mw3": [128, 64],
    "mb1c": [64, 1], "mb2c": [128, 1], "mb3c": [64, 1],
}
# tensors that feed PE matmuls (get the float32r dtype)
_MM_STAT = {"w1aug", "brow", "wb2", "wb3", "wb4", "b8bc", "w5b", "onesr"}


# --------------------------------------------------------------- device side

def build_program(cfg=None):
    import concourse.bacc as bacc
    import concourse.mybir as mybir
    from concourse.ap import AP
    from concourse.tile import TileContext

    cfg = dict(CFG if cfg is None else cfg)
    f32 = mybir.dt.float32
    mdt = {"fp16": mybir.dt.float16, "f32r": mybir.dt.float32r,
           "f32": f32}[cfg["mm_dt"]]
    AF = mybir.ActivationFunctionType
    OP = mybir.AluOpType
    AF_LREL = AF.Prelu if cfg["prelu"] else AF.Lrelu

    class _KBacc(bacc.Bacc):
        # The stock insert_act_table_loads greedily picks the FIRST table
        # set containing each activation function, which alternates sets
        # for a Square/Sqrt/Prelu mix -> a ~2.7us ACT_TABLE_LOAD per
        # transition.  Everything we use lives in one set, so blank all
        # other sets (list positions = set ids must be preserved).
        _ACT_SET = "abs_reciprocal_sqrt_and_small"

        def insert_act_table_loads(self):
            import concourse.mybir as _mb
            from concourse.hw_specs import get_activation_tables
            has_activation = any(
                isinstance(i, _mb.InstActivation)
                for b in self.main_func.blocks
                for i in b.instructions
            )
            if not has_activation:
                return
            tables = []
            for name, funcs in get_activation_tables(self.m.arch).items():
                tables.append((name, funcs if name == self._ACT_SET else set()))
            bacc._bass_rust.insert_act_table_loads(self, tables)

    nc = _KBacc("TRN2", target_bir_lowering=False)
    x_d = nc.dram_tensor("x", [BS, T], mdt, kind="ExternalInput")
    md_d = nc.dram_tensor("metadata", [BS, F], f32, kind="ExternalInput")
    y_d = nc.dram_tensor("y", [BS, T], f32, kind="ExternalOutput")
    wd = {name: nc.dram_tensor(name, shp, mdt if name in _MM_STAT else f32,
                               kind="ExternalInput")
          for name, shp in _WSHAPES.items()}

    with TileContext(nc) as tc:
        with tc.tile_pool(name="const", bufs=1) as cp, \
             tc.tile_pool(name="scr", bufs=1, space="DRAM") as dp:

            # ---- constants to SBUF
            cw = {}
            for name, shp in _WSHAPES.items():
                t = cp.tile(shp, mdt if name in _MM_STAT else f32,
                            name=f"c_{name}")
                nc.sync.dma_start(out=t[:], in_=wd[name][:])
                cw[name] = t
            eps_s = cp.tile([128, 1], f32, name="eps_s")
            nc.vector.memset(eps_s[:], 1e-12)

            # ---- style MLP (per-core 32 samples)
            with tc.tile_pool(name="stp", bufs=1, space="PSUM") as sp:
                mdT = cp.tile([F, BS], f32, name="mdT")
                nc.sync.dma_start(out=mdT[:], in_=md_d.rearrange("s f -> f s"))
                ps1 = sp.tile([64, BS], f32, name="ps1")
                nc.tensor.matmul(ps1[:], cw["mw1"][:], mdT[:],
                                 start=True, stop=True)
                s1 = cp.tile([64, BS], f32, name="s1")
                nc.scalar.activation(s1[:], ps1[:], AF.Relu, bias=cw["mb1c"][:])
                ps2 = sp.tile([128, BS], f32, name="ps2")
                nc.tensor.matmul(ps2[:], cw["mw2"][:], s1[:],
                                 start=True, stop=True)
                s2 = cp.tile([128, BS], f32, name="s2")
                nc.scalar.activation(s2[:], ps2[:], AF.Relu, bias=cw["mb2c"][:])
                ps3 = sp.tile([64, BS], f32, name="ps3")
                nc.tensor.matmul(ps3[:], cw["mw3"][:], s2[:],
                                 start=True, stop=True)
                sT = cp.tile([64, BS], f32, name="sT")
                nc.scalar.activation(sT[:], ps3[:], AF.Identity,
                                     bias=cw["mb3c"][:])

            # ---- per-(layer, supergroup) scale/bias vectors via DRAM trip
            # sT row = 16(l-1) + 2c + (0 scale / 1 bias), col = 16 sg + s'
            sT_d = dp.tile([64, BS], f32, name="sT_d")
            nc.gpsimd.dma_start(out=sT_d[:], in_=sT[:])
            scv = cp.tile([128, 8], f32, name="scv")   # scale, col j=(l-1)*2+sg
            bcv = cp.tile([128, 8], f32, name="bcv")   # bias
            for l in range(1, 5):
                for g in range(2):
                    j = (l - 1) * 2 + g
                    src_s = AP(tensor=sT_d[:].tensor,
                               offset=512 * (l - 1) + 16 * g,
                               ap=((1, 16), (64, 8)))
                    nc.gpsimd.dma_start(out=scv[:, j:j + 1], in_=src_s)
                    src_b = AP(tensor=sT_d[:].tensor,
                               offset=512 * (l - 1) + 32 + 16 * g,
                               ap=((1, 16), (64, 8)))
                    nc.gpsimd.dma_start(out=bcv[:, j:j + 1], in_=src_b)

            # ---------------- main loop
            with tc.tile_pool(name="pd", bufs=3, space="PSUM") as pdp, \
                 tc.tile_pool(name="pv", bufs=1, space="PSUM") as pvp, \
                 tc.tile_pool(name="xin", bufs=3) as xp, \
                 tc.tile_pool(name="dsqp", bufs=3) as dqp, \
                 tc.tile_pool(name="sgp", bufs=2) as sgp, \
                 tc.tile_pool(name="rpool", bufs=3) as rpp, \
                 tc.tile_pool(name="qpool", bufs=3) as qpp, \
                 tc.tile_pool(name="actp", bufs=3) as app, \
                 tc.tile_pool(name="outp", bufs=3) as opp:

                for g in range(2):
                    for k in range(4):
                        xt = xp.tile([17, 2048], mdt, name="xt", tag="xt")
                        nc.sync.dma_start(
                            out=xt[0:16, :],
                            in_=x_d[16 * g:16 * g + 16,
                                    2048 * k:2048 * (k + 1)])
                        nc.sync.dma_start(out=xt[16:17, :],
                                          in_=cw["onesr"][:])
                        a_prev = None
                        for l in range(1, 5):
                            j = (l - 1) * 2 + g
                            prs = [pdp.tile([128, 1024], f32,
                                            name=f"dt{l}{p}", tag="dt")
                                   for p in range(2)]

                            def dtap(tau):
                                h = tau % 2
                                return prs[tau // 2][:, 512 * h:512 * (h + 1)]

                            for tau in range(4):
                                sl = slice(512 * tau, 512 * (tau + 1))
                                if l == 1:
                                    nc.tensor.matmul(dtap(tau),
                                                     cw["w1aug"][:],
                                                     xt[:, sl],
                                                     start=True, stop=True)
                                else:
                                    nc.tensor.matmul(dtap(tau),
                                                     cw[f"wb{l}"][:],
                                                     a_prev[:, sl],
                                                     start=True, stop=False)
                                    bsl = slice(128 * (l - 2), 128 * (l - 1))
                                    nc.tensor.matmul(dtap(tau),
                                                     cw["brow"][:, bsl],
                                                     cw["onesr"][:, 0:512],
                                                     start=False, stop=True)
                            dsq = dqp.tile([128, 2048], mdt,
                                           name=f"dsq{l}", tag="dsq")
                            for p in range(2):
                                nc.scalar.activation(
                                    dsq[:, 1024 * p:1024 * (p + 1)],
                                    prs[p][:], AF.Square)
                            anew = app.tile([128, 2048], mdt,
                                            name=f"a{l}", tag="a")
                            for p in range(2):
                                psl = slice(1024 * p, 1024 * (p + 1))
                                vb = pvp.tile([128, 1024], f32,
                                              name=f"vb{l}{p}", tag="vb")
                                for h in range(2):
                                    tau = 2 * p + h
                                    nc.tensor.matmul(
                                        vb[:, 512 * h:512 * (h + 1)],
                                        cw["b8bc"][:],
                                        dsq[:, 512 * tau:512 * (tau + 1)],
                                        start=True, stop=True)
                                # r = 1/sigma = (|var/7 + eps|)^-1/2
                                r_ = rpp.tile([128, 1024], f32,
                                              name=f"r{l}{p}", tag="r")
                                nc.scalar.activation(r_[:], vb[:],
                                                     AF.Abs_reciprocal_sqrt,
                                                     scale=1.0 / 7.0,
                                                     bias=eps_s[:])
                                q_ = qpp.tile([128, 1024], f32,
                                              name=f"q{l}{p}", tag="q")
                                nc.vector.tensor_tensor(q_[:], prs[p][:],
                                                        r_[:], OP.mult)
                                nc.scalar.activation(
                                    anew[:, psl], q_[:], AF_LREL,
                                    scale=scv[:, j:j + 1],
                                    bias=bcv[:, j:j + 1], alpha=0.01)
                            a_prev = anew

                        # ---- L5 (no adain): packed [64,512] output
                        h5t = pdp.tile([128, 1024], f32, name="h5", tag="dt")
                        h5 = h5t[0:64, 0:512]
                        for tau in range(4):
                            sl = slice(512 * tau, 512 * (tau + 1))
                            nc.tensor.matmul(
                                h5, cw["w5b"][:, 64 * tau:64 * (tau + 1)],
                                a_prev[:, sl],
                                start=(tau == 0), stop=(tau == 3))
                        oc = opp.tile([64, 512], f32, name="oc", tag="oc")
                        nc.scalar.activation(oc[:], h5, AF_LREL,
                                             bias=cw["b5c"][:], alpha=0.01)
                        ydst = y_d.rearrange(
                            "(sg sp) (kk tau n) -> sg kk tau sp n",
                            sg=2, kk=4, tau=4, n=512)[g, k]
                        # oc partition-major order (p = 16 tau + sp) matches
                        # the (tau, sp, n) iteration of ydst
                        nc.sync.dma_start(out=ydst, in_=oc[:])

    nc.compile()
    return nc


# ------------------------------------------------------------------- runner

def _get_program():
    key = tuple(sorted(CFG.items()))
    if key not in _CACHE:
        _CACHE[key] = build_program(CFG)
    return _CACHE[key]


def _make_in_maps(inputs):
    mm_np = {"fp16": np.float16, "f32r": np.float32, "f32": np.float32}[
        CFG["mm_dt"]]
    x = np.ascontiguousarray(
        np.asarray(inputs["x"], dtype=np.float32).reshape(B_FULL, T).astype(
            mm_np))
    md = np.ascontiguousarray(np.asarray(inputs["metadata"], dtype=np.float32))
    wts = _host_weights(inputs)
    in_maps = []
    for i in range(NCORES):
        m = dict(wts)
        m["x"] = np.ascontiguousarray(x[BS * i:BS * (i + 1)])
        m["metadata"] = np.ascontiguousarray(md[BS * i:BS * (i + 1)])
        in_maps.append(m)
    return in_maps


def run_spmd(inputs, trace=False):
    """Run on all 8 cores; returns (y_full, BassKernelResults)."""
    from concourse.bass_utils import run_bass_kernel_spmd
    nc = _get_program()
    in_maps = _make_in_maps(inputs)
    res = run_bass_kernel_spmd(nc, in_maps, core_ids=list(range(NCORES)),
                               trace=trace)
    y = np.concatenate([np.asarray(r["y"]) for r in res.results], axis=0)
    y = y.reshape(B_FULL, 1, T).astype(np.float32)
    return y, res


def kernel(**inputs):
    y, _ = run_spmd(inputs, trace=False)
    return y
